# revision 22
# baseline (speedup 1.0000x reference)
"""Multi-head attention (B=2, S=2048, D=1024, H=16) on 8 Trainium2 cores.

Sharding: core = (batch b, head-group g): 2 batches x 4 groups of 4 heads.
Each core computes Q/K/V projections for its 256 model columns, causal
attention for its 4 heads, and a partial output projection through its
256 rows of Wo. Host sums the 4 partials per batch (the "all-reduce").

Device-side layout strategy (per core):
  - Host passes query/key/value pre-tiled+transposed: [NSB, 128, 8, 512]
    (contiguous 8KB-per-partition DMA runs, one descriptor set per slab).
  - QT/KT [c=256, s] produced directly with W stationary (full-speed MMs).
  - V [s, c] produced with xT stationary, padded with a ones column per
    head so the attnV matmul also yields the softmax denominator l.
  - Scores computed transposed: ST[k, q], one psum tile per (head-pair, j)
    holding both heads (row-disjoint matmuls overlap in the PE array);
    additive causal mask on the diagonal 128-blocks; q range trimmed to
    [128*r, 512) on diagonal blocks; exp on ScalarE with fused 1/sqrt(64)
    scale (max-subtraction skipped: scores bounded).
  - attnV: outT[d(+l), q] = V_aug^T @ PT, accumulated over k blocks in
    PSUM; columns below the causal diagonal are skipped entirely.
  - Normalize with reciprocal_approx_fast + GpSimd partition_broadcast;
    the final scale reads PSUM directly (no staging copy).
  - Output projection: lhsT = OT chunks, rhs = Wo -> partial out [s, e],
    written back as bf16 (tolerance allows it; halves the write traffic).

Global schedule (causal): attention q-blocks processed in order 1,2,3,0.
The exp on ScalarE is the throughput limit inside an attention block
(1.15us per 128-k-block vs 0.64us of PE work), so projection and
output-projection chunks are pinned at explicit positions inside each
attention block to keep the PE busy during exp gaps, weighted so each
phase's PE filler matches its ScalarE time. Processing block 0 (all
diagonal, tiny exp cost) last keeps the tail short and the PE warm.
All matmuls use bf16 operands (full PE speed).
"""

import os
import numpy as np
from contextlib import ExitStack

import concourse.bass as bass
import concourse.tile as tile
from concourse import bacc, mybir
from concourse import bass_utils
from concourse.bass import ts

B, S, D, H = 2, 2048, 1024, 16
DEPTH = D // H            # 64
NCORES = 8
GROUPS = 4                # head-groups per batch
HG = H // GROUPS          # 4 heads per core
CW = HG * DEPTH           # 256 local columns
P = 128
DC = D // P               # 8 contraction chunks
NST = S // P              # 16 seq tiles of 128
NSB = S // 512            # 4 seq blocks of 512
F32 = mybir.dt.float32
FR = mybir.dt.bfloat16
SCALE = 1.0 / float(np.sqrt(DEPTH))  # 0.125
NEG = np.float32(-1e9 / SCALE)


def _build_program(mode, use_q_bias, use_k_bias, use_v_bias):
    """mode: 'causal' | 'dense' | 'generic'."""
    nc = bacc.Bacc(
        "TRN2",
        target_bir_lowering=False,
        debug=False,
        enable_asserts=False,
        num_devices=NCORES,
    )

    xq = nc.dram_tensor("xq", [NSB, P, DC, 512], FR, kind="ExternalInput").ap()
    xk = nc.dram_tensor("xk", [NSB, P, DC, 512], FR, kind="ExternalInput").ap()
    xv = nc.dram_tensor("xv", [NSB, P, DC, 512], FR, kind="ExternalInput").ap()
    wq = nc.dram_tensor("wq", [P, DC, CW], FR, kind="ExternalInput").ap()
    wk = nc.dram_tensor("wk", [P, DC, CW], FR, kind="ExternalInput").ap()
    wv = nc.dram_tensor("wv", [P, DC, CW], FR, kind="ExternalInput").ap()
    wo = nc.dram_tensor("wo", [P, CW // P, D], FR, kind="ExternalInput").ap()
    mtri = None
    mneg = None
    if mode == "causal":
        mtri = nc.dram_tensor("mtri", [P, P], F32, kind="ExternalInput").ap()
    elif mode == "generic":
        mneg = nc.dram_tensor("mneg", [S, S], F32, kind="ExternalInput").ap()
    bq = bk = bv = None
    if use_q_bias:
        bq = nc.dram_tensor("bq", [P, CW // P], F32, kind="ExternalInput").ap()
    if use_k_bias:
        bk = nc.dram_tensor("bk", [P, CW // P], F32, kind="ExternalInput").ap()
    if use_v_bias:
        bv = nc.dram_tensor("bv", [P, CW], F32, kind="ExternalInput").ap()
    # [NST, P, 2, 512] has the same memory layout as [S, D] row-major
    out = nc.dram_tensor("out", [NST, P, 2, 512], FR, kind="ExternalOutput").ap()

    with tile.TileContext(nc) as tc, ExitStack() as ctx:
        wpool = ctx.enter_context(tc.tile_pool(name="wpool", bufs=1))
        xpool = ctx.enter_context(tc.tile_pool(name="xpool", bufs=6))
        qkpool = ctx.enter_context(tc.tile_pool(name="qkpool", bufs=1))
        ptpool = ctx.enter_context(tc.tile_pool(name="ptpool", bufs=6))
        smpool = ctx.enter_context(tc.tile_pool(name="smpool", bufs=4))
        outpool = ctx.enter_context(tc.tile_pool(name="outpool", bufs=3))
        mkpool = ctx.enter_context(tc.tile_pool(name="mkpool", bufs=3))
        # PSUM: pf (proj + final, 2x1 bank) + ps (scores pairs, 2x2 banks)
        # + po (attnV accum, 2x1 bank) = 8 banks exactly
        pf = ctx.enter_context(tc.tile_pool(name="pf", bufs=2, space="PSUM"))
        ps = ctx.enter_context(tc.tile_pool(name="ps", bufs=2, space="PSUM"))
        po = ctx.enter_context(tc.tile_pool(name="po", bufs=2, space="PSUM"))

        # --- persistent SBUF tensors (DMAs emitted lazily in the stream) ---
        wq_sb = wpool.tile([P, DC, CW], FR, tag="wq_sb")
        wk_sb = wpool.tile([P, DC, CW], FR, tag="wk_sb")
        wv_sb = wpool.tile([P, DC, CW], FR, tag="wv_sb")
        wo_sb = wpool.tile([P, CW // P, D], FR, tag="wo_sb")
        w_dma = {
            "q": lambda: nc.sync.dma_start(wq_sb[:], wq),
            "k": lambda: nc.sync.dma_start(wk_sb[:], wk),
            "v": lambda: nc.sync.dma_start(wv_sb[:], wv),
            "o": lambda: nc.sync.dma_start(wo_sb[:], wo),
        }
        mtri_sb = None
        if mode == "causal":
            mtri_sb = wpool.tile([P, P], F32, tag="mtri_sb")
        ones_v = wpool.tile([P, HG, 1], F32, tag="ones_v")
        nc.vector.memset(ones_v[:], 1.0)
        bq_sb = bk_sb = bv_sb = None
        if use_q_bias:
            bq_sb = wpool.tile([P, CW // P], F32, tag="bq_sb")
            nc.sync.dma_start(bq_sb[:], bq)
        if use_k_bias:
            bk_sb = wpool.tile([P, CW // P], F32, tag="bk_sb")
            nc.sync.dma_start(bk_sb[:], bk)
        if use_v_bias:
            bv_sb = wpool.tile([P, CW], F32, tag="bv_sb")
            nc.sync.dma_start(bv_sb[:], bv)

        # Persistent per-block result tiles (fine-grained deps).
        QT_t = {}  # (cc, sb) -> [128, 512]
        KT_t = {}
        OT_t = {}
        for cc in range(CW // P):
            for sb in range(NSB):
                QT_t[(cc, sb)] = qkpool.tile(
                    [P, 512], FR, name=f"qt_{cc}_{sb}", tag=f"qt_{cc}_{sb}")
                KT_t[(cc, sb)] = qkpool.tile(
                    [P, 512], FR, name=f"kt_{cc}_{sb}", tag=f"kt_{cc}_{sb}")
                OT_t[(cc, sb)] = qkpool.tile(
                    [P, 512], FR, name=f"ot_{cc}_{sb}", tag=f"ot_{cc}_{sb}")
        V_t = {}  # st -> [128, HG, DEPTH+1] (ones col per head)
        for st in range(NST):
            V_t[st] = qkpool.tile(
                [P, HG, DEPTH + 1], FR, name=f"v_{st}", tag=f"v_{st}")

        slabs = {}  # (nm, sl) -> slab tile
        x_of = {"q": xq, "k": xk, "v": xv}

        def load_slab(nm, sl):
            def _c():
                slab = xpool.tile([P, DC, 512], FR, tag="slab",
                                  name=f"sl{nm}_{sl}")
                # two half-slab DMAs: the first 4 dc chunks arrive early so
                # the projection's accumulation chain can start sooner
                h = DC // 2
                nc.sync.dma_start(slab[:, 0:h, :], x_of[nm][sl][:, 0:h, :])
                nc.sync.dma_start(slab[:, h:, :], x_of[nm][sl][:, h:, :])
                slabs[(nm, sl)] = slab
            return _c

        def proj_psum(pool_sel, name):
            # a [P,512] accumulator from either psum pool; the "ps" pool's
            # slot is 2 banks wide, use its first bank
            if pool_sel == "ps":
                t = ps.tile([P, 2, 512], F32, tag="ps", name=name)
                return t[:, 0, :]
            return pf.tile([P, 512], F32, tag="pf", name=name)

        def v_group(sl, sq, pool_sel="pf"):
            def _c():
                st = sl * 4 + sq
                slab = slabs[("v", sl)]
                psum_v = proj_psum(pool_sel, f"pv_{st}")
                for dc in range(DC):
                    nc.tensor.matmul(
                        psum_v[:, :CW],
                        lhsT=slab[:, dc, ts(sq, P)],
                        rhs=wv_sb[:, dc, :],
                        start=(dc == 0),
                        stop=(dc == DC - 1),
                    )
                psrc = psum_v[:, :CW].rearrange("p (h d) -> p h d", h=HG)
                if use_v_bias:
                    nc.vector.tensor_tensor(
                        V_t[st][:, :, 0:DEPTH], psrc,
                        bv_sb.rearrange("p (h d) -> p h d", h=HG),
                        mybir.AluOpType.add,
                    )
                else:
                    nc.vector.tensor_copy(V_t[st][:, :, 0:DEPTH], psrc)
                nc.vector.tensor_copy(
                    V_t[st][:, :, DEPTH : DEPTH + 1], ones_v[:])
            return _c

        def qk_group(nm, cc, sl, pool_sel="pf"):
            w_sb, b_sb, T_t = {
                "q": (wq_sb, bq_sb, QT_t),
                "k": (wk_sb, bk_sb, KT_t),
            }[nm]

            def _c():
                slab = slabs[(nm, sl)]
                psum_q = proj_psum(pool_sel, f"p{nm}_{cc}_{sl}")
                for dc in range(DC):
                    nc.tensor.matmul(
                        psum_q[:],
                        lhsT=w_sb[:, dc, ts(cc, P)],
                        rhs=slab[:, dc, :],
                        start=(dc == 0),
                        stop=(dc == DC - 1),
                    )
                if b_sb is not None:
                    nc.vector.tensor_scalar_add(
                        T_t[(cc, sl)][:], psum_q[:], b_sb[:, cc : cc + 1])
                else:
                    nc.vector.tensor_copy(T_t[(cc, sl)][:], psum_q[:])
            return _c

        def attention_block(i, inject=(), l_on_scalar=False):
            """inject: list of (pos, chunk); chunk emitted when the unit
            counter (cc-major over j) reaches pos."""
            inject = sorted(inject, key=lambda t: t[0])
            inj_idx = 0
            jmax = 4 * i + 4 if mode == "causal" else NST
            jcount = 0
            for cc in range(CW // P):  # head pair (2cc, 2cc+1)
                po0 = po.tile([DEPTH + 1, 512], F32, tag="po",
                              name=f"po0_{i}_{cc}")
                po1 = po.tile([DEPTH + 1, 512], F32, tag="po",
                              name=f"po1_{i}_{cc}")
                pos = (po0, po1)
                for j in range(jmax):
                    while inj_idx < len(inject) and inject[inj_idx][0] <= jcount:
                        inject[inj_idx][1]()
                        inj_idx += 1
                    r = j - 4 * i
                    lo = P * r if (mode == "causal" and r >= 0) else 0
                    psj = ps.tile([P, 2, 512], F32, tag="ps",
                                  name=f"ps_{i}_{cc}_{j}")
                    for hh in range(2):
                        nc.tensor.matmul(
                            psj[:, hh, lo:],
                            lhsT=KT_t[(cc, j // 4)][
                                DEPTH * hh : DEPTH * hh + DEPTH, ts(j % 4, P)],
                            rhs=QT_t[(cc, i)][DEPTH * hh : DEPTH * hh + DEPTH, lo:],
                            start=True,
                            stop=True,
                        )
                    if mode == "causal" and r >= 0:
                        nc.vector.tensor_tensor(
                            psj[:, :, lo : lo + P],
                            psj[:, :, lo : lo + P],
                            mtri_sb[:, None, :].to_broadcast((P, 2, P)),
                            mybir.AluOpType.add,
                        )
                    elif mode == "generic":
                        mk = mkpool.tile([P, 512], F32, tag="mk",
                                         name=f"mk_{i}_{cc}_{j}")
                        nc.sync.dma_start(mk[:], mneg[ts(j, P), ts(i, 512)])
                        nc.vector.tensor_tensor(
                            psj[:], psj[:],
                            mk[:, None, :].to_broadcast((P, 2, 512)),
                            mybir.AluOpType.add,
                        )
                    pt = ptpool.tile([P, 2, 512], FR, tag="pt",
                                     name=f"pt_{i}_{cc}_{j}")
                    nc.scalar.activation(
                        pt[:, :, lo:],
                        psj[:, :, lo:],
                        mybir.ActivationFunctionType.Exp,
                        scale=SCALE,
                    )
                    for hh in range(2):
                        nc.tensor.matmul(
                            pos[hh][:, lo:],
                            lhsT=V_t[j][:, 2 * cc + hh, :],
                            rhs=pt[:, hh, lo:],
                            start=(j == 0),
                            stop=(j == jmax - 1),
                        )
                    jcount += 1
                # normalize both heads: OT[c, q] = outT[c, q] / l[q].
                # l staged to SBUF (custom-DVE recip can't read PSUM); the
                # final scale reads the PSUM accumulator directly.
                l_sb = {}
                rl_sb = {}
                rb = {}
                for hh in range(2):
                    l_sb[hh] = smpool.tile([1, 512], F32, tag="l_sb",
                                           name=f"l_{i}_{cc}_{hh}")
                    if l_on_scalar:
                        nc.scalar.copy(
                            l_sb[hh][:], pos[hh][DEPTH : DEPTH + 1, :])
                    else:
                        nc.vector.tensor_copy(
                            l_sb[hh][:], pos[hh][DEPTH : DEPTH + 1, :])
                for hh in range(2):
                    rl_sb[hh] = smpool.tile([1, 512], F32, tag="rl_sb",
                                            name=f"rl_{i}_{cc}_{hh}")
                    nc.vector.reciprocal_approx_fast(
                        out=rl_sb[hh][:], in_=l_sb[hh][:])
                for hh in range(2):
                    rb[hh] = smpool.tile([DEPTH, 512], F32, tag="rb",
                                         name=f"rb_{i}_{cc}_{hh}")
                    nc.gpsimd.partition_broadcast(rb[hh][:], rl_sb[hh][:])
                for hh in range(2):
                    nc.vector.tensor_tensor(
                        OT_t[(cc, i)][DEPTH * hh : DEPTH * hh + DEPTH, :],
                        pos[hh][0:DEPTH, :],
                        rb[hh][:],
                        mybir.AluOpType.mult,
                    )

            while inj_idx < len(inject):
                inject[inj_idx][1]()
                inj_idx += 1

        def output_chunks(i, split_engines=False, alt_pool=False):
            """8 chunks; each = 2 accumulating fin MMs + copy; one DMA per
            qq once both eh halves are in SBUF."""
            chunks = []
            out_ts = {}

            def fin_group(qq, eh, i=i):
                def _c():
                    qt = 4 * i + qq
                    psum_f = proj_psum(
                        "ps" if (alt_pool and eh == 0) else "pf",
                        f"pfin_{qt}_{eh}")
                    for cc2 in range(CW // P):
                        nc.tensor.matmul(
                            psum_f[:],
                            lhsT=OT_t[(cc2, i)][:, ts(qq, P)],
                            rhs=wo_sb[:, cc2, ts(eh, 512)],
                            start=(cc2 == 0),
                            stop=(cc2 == CW // P - 1),
                        )
                    if eh == 0:
                        out_ts[qq] = outpool.tile(
                            [P, 2, 512], FR, tag="out_t", name=f"ot_{qt}")
                    out_t = out_ts[qq]
                    if split_engines and eh == 1:
                        nc.scalar.copy(out_t[:, eh, :], psum_f[:])
                    else:
                        nc.vector.tensor_copy(out_t[:, eh, :], psum_f[:])
                    if eh == 1:
                        nc.sync.dma_start(out[qt], out_t[:])
                return _c

            for qq in range(4):
                for eh in range(2):
                    chunks.append(fin_group(qq, eh))
            return chunks

        def output_block(i, split_engines=False, alt_pool=False):
            for c in output_chunks(i, split_engines, alt_pool):
                c()

        if mode == "causal":
            # -- pre-phase: minimum projections for attention block 1 --
            w_dma["k"]()
            load_slab("k", 0)()
            w_dma["q"]()
            load_slab("q", 1)()
            w_dma["v"]()
            load_slab("v", 0)()
            nc.sync.dma_start(mtri_sb[:], mtri)
            qk_group("k", 0, 0, "ps")()
            qk_group("k", 1, 0, "pf")()
            qk_group("q", 0, 1, "ps")()
            qk_group("q", 1, 1, "pf")()
            for sq in range(4):
                v_group(0, sq, "ps" if sq % 2 == 0 else "pf")()

            # -- attention phases with positioned PE filler --
            out1 = output_chunks(1)
            out2 = output_chunks(2)
            attention_block(1, inject=[
                (0, load_slab("k", 1)), (0, load_slab("v", 1)),
                (1, qk_group("k", 0, 1)), (2, qk_group("k", 1, 1)),
                (3, v_group(1, 0)), (4, v_group(1, 1)),
                (5, v_group(1, 2)), (6, v_group(1, 3)),
                (7, load_slab("q", 2)), (8, w_dma["o"]),
                (9, load_slab("k", 2)), (11, load_slab("v", 2)),
                (13, qk_group("q", 0, 2)),
            ])
            attention_block(2, inject=[
                (0, qk_group("q", 1, 2)),
                (1, qk_group("k", 0, 2)), (3, qk_group("k", 1, 2)),
                (4, v_group(2, 0)), (5, v_group(2, 1)),
                (6, v_group(2, 2)), (7, v_group(2, 3)),
                (9, load_slab("q", 3)), (10, qk_group("q", 0, 3)),
                (12, qk_group("q", 1, 3)),
                (14, load_slab("k", 3)), (19, load_slab("v", 3)),
            ])
            attention_block(3, inject=[
                (0, load_slab("q", 0)),
                (1, qk_group("k", 0, 3)), (3, qk_group("k", 1, 3)),
                (2, v_group(3, 0)), (4, v_group(3, 1)),
                (6, v_group(3, 2)), (8, v_group(3, 3)),
                (10, qk_group("q", 0, 0)), (12, qk_group("q", 1, 0)),
                (14, out1[0]), (16, out1[1]), (18, out1[2]),
                (20, out1[3]), (22, out1[4]), (24, out1[5]),
                (26, out1[6]), (28, out1[7]),
            ])
            # out2's fins pinned at the two po-ring stalls (positions 0 and
            # 4): the PE blocks there on the previous unit's normalize, and
            # filler behind the blocked attnV in the queue cannot run
            attention_block(0, inject=[
                (0, out2[0]), (0, out2[1]), (0, out2[2]), (0, out2[3]),
                (4, out2[4]), (4, out2[5]), (4, out2[6]), (4, out2[7]),
            ], l_on_scalar=True)
            # tail: out3 first (independent of block 0's normalize, hides
            # that chain), then out0; both pipelined across the two psum
            # pools and with copies split across Vector/Scalar engines
            output_block(3, split_engines=True, alt_pool=True)
            output_block(0, split_engines=True, alt_pool=True)
        else:
            # dense/generic need all KT/V before any attention block
            w_dma["o"]()
            for nm in ("v", "q", "k"):
                w_dma[nm]()
            for sl in range(NSB):
                load_slab("v", sl)()
                for sq in range(4):
                    v_group(sl, sq)()
                load_slab("q", sl)()
                for cc in range(CW // P):
                    qk_group("q", cc, sl)()
                load_slab("k", sl)()
                for cc in range(CW // P):
                    qk_group("k", cc, sl)()
            for i in range(NSB):
                attention_block(i)
                output_block(i)

    nc.compile()
    return nc


_PROG_CACHE = {}


def _get_program(mode, use_q_bias, use_k_bias, use_v_bias):
    key = (mode, use_q_bias, use_k_bias, use_v_bias)
    if key not in _PROG_CACHE:
        _PROG_CACHE[key] = _build_program(mode, use_q_bias, use_k_bias, use_v_bias)
    return _PROG_CACHE[key]


import ml_dtypes


def _pretile(x2d):
    # [S, D] -> [NSB, P, DC, 512]: arr[sl, p, dc, s] = x2d[sl*512+s, dc*128+p]
    return np.ascontiguousarray(
        x2d.reshape(NSB, 512, DC, P).transpose(0, 3, 2, 1)
    ).astype(ml_dtypes.bfloat16)


def _pretile_w(w):
    # [D, CW] -> [P, DC, CW]
    return np.ascontiguousarray(
        w.reshape(DC, P, CW).transpose(1, 0, 2)).astype(ml_dtypes.bfloat16)


def kernel(**inputs):
    query = np.asarray(inputs["query"], np.float32)
    key = np.asarray(inputs["key"], np.float32)
    value = np.asarray(inputs["value"], np.float32)
    mask = np.asarray(inputs["mask"], np.float32).reshape(S, S)
    wq = np.asarray(inputs["wq"], np.float32)
    wk = np.asarray(inputs["wk"], np.float32)
    wv = np.asarray(inputs["wv"], np.float32)
    wo = np.asarray(inputs["wo"], np.float32)
    bq = np.asarray(inputs["bq"], np.float32)
    bk = np.asarray(inputs["bk"], np.float32)
    bv = np.asarray(inputs["bv"], np.float32)
    bo = np.asarray(inputs["bo"], np.float32)

    if not mask.any():
        mode = "dense"
    elif np.array_equal(mask, np.triu(np.ones((S, S), np.float32), 1)):
        mode = "causal"
    else:
        mode = "generic"
    use_q_bias = bool(bq.any())
    use_k_bias = bool(bk.any())
    use_v_bias = bool(bv.any())

    nc = _get_program(mode, use_q_bias, use_k_bias, use_v_bias)

    in_maps = []
    for core in range(NCORES):
        b, g = core // GROUPS, core % GROUPS
        cs = slice(g * CW, (g + 1) * CW)
        m = {
            "xq": _pretile(query[b]),
            "xk": _pretile(key[b]),
            "xv": _pretile(value[b]),
            "wq": _pretile_w(wq[:, cs]),
            "wk": _pretile_w(wk[:, cs]),
            "wv": _pretile_w(wv[:, cs]),
            "wo": np.ascontiguousarray(
                wo[cs, :].reshape(CW // P, P, D).transpose(1, 0, 2)
            ).astype(ml_dtypes.bfloat16),
        }
        if mode == "causal":
            m["mtri"] = np.where(
                np.triu(np.ones((P, P), bool), 0), np.float32(0), NEG
            ).astype(np.float32)
        elif mode == "generic":
            m["mneg"] = np.ascontiguousarray(mask.T) * NEG
        if use_q_bias:
            m["bq"] = np.ascontiguousarray(bq[cs].reshape(CW // P, P).T)
        if use_k_bias:
            m["bk"] = np.ascontiguousarray(bk[cs].reshape(CW // P, P).T)
        if use_v_bias:
            m["bv"] = np.ascontiguousarray(np.tile(bv[cs], (P, 1)))
        in_maps.append(m)

    res = bass_utils.run_bass_kernel_spmd(
        nc, in_maps, core_ids=list(range(NCORES)), trace=False
    )
    outs = [np.asarray(r["out"], np.float32).reshape(S, D) for r in res.results]
    full = np.empty((B, S, D), np.float32)
    for b in range(B):
        full[b] = outs[GROUPS * b]
        for g in range(1, GROUPS):
            full[b] += outs[GROUPS * b + g]
        full[b] += bo
    return full


# revision 26
# speedup vs baseline: 1.0401x; 1.0401x over previous
"""Multi-head attention (B=2, S=2048, D=1024, H=16) on 8 Trainium2 cores.

Sharding: core = (batch b, head-group g): 2 batches x 4 groups of 4 heads.
Each core computes Q/K/V projections for its 256 model columns, causal
attention for its 4 heads, and a partial output projection through its
256 rows of Wo. Host sums the 4 partials per batch (the "all-reduce").

Device-side layout strategy (per core):
  - Host passes query/key/value pre-tiled+transposed: [NSB, 128, 8, 512]
    (contiguous 8KB-per-partition DMA runs, one descriptor set per slab).
  - QT/KT [c=256, s] produced directly with W stationary (full-speed MMs).
  - V [s, c] produced with xT stationary, padded with a ones column per
    head so the attnV matmul also yields the softmax denominator l.
  - Scores computed transposed: ST[k, q], one psum tile per (head-pair, j)
    holding both heads (row-disjoint matmuls overlap in the PE array);
    additive causal mask on the diagonal 128-blocks; q range trimmed to
    [128*r, 512) on diagonal blocks; exp on ScalarE with fused 1/sqrt(64)
    scale (max-subtraction skipped: scores bounded).
  - attnV: outT[d(+l), q] = V_aug^T @ PT, accumulated over k blocks in
    PSUM; columns below the causal diagonal are skipped entirely.
  - Normalize with reciprocal_approx_fast + GpSimd partition_broadcast;
    the final scale reads PSUM directly (no staging copy).
  - Output projection: lhsT = OT chunks, rhs = Wo -> partial out [s, e],
    written back as bf16 (tolerance allows it; halves the write traffic).

Global schedule (causal): attention q-blocks processed in order 1,2,3,0.
The exp on ScalarE is the throughput limit inside an attention block
(1.15us per 128-k-block vs 0.64us of PE work), so projection and
output-projection chunks are pinned at explicit positions inside each
attention block to keep the PE busy during exp gaps, weighted so each
phase's PE filler matches its ScalarE time. Processing block 0 (all
diagonal, tiny exp cost) last keeps the tail short and the PE warm.
All matmuls use bf16 operands (full PE speed).
"""

import os
import numpy as np
from contextlib import ExitStack

import concourse.bass as bass
import concourse.tile as tile
from concourse import bacc, mybir
from concourse import bass_utils
from concourse.bass import ts

B, S, D, H = 2, 2048, 1024, 16
DEPTH = D // H            # 64
NCORES = 8
GROUPS = 4                # head-groups per batch
HG = H // GROUPS          # 4 heads per core
CW = HG * DEPTH           # 256 local columns
P = 128
DC = D // P               # 8 contraction chunks
NST = S // P              # 16 seq tiles of 128
NSB = S // 512            # 4 seq blocks of 512
F32 = mybir.dt.float32
FR = mybir.dt.bfloat16
SCALE = 1.0 / float(np.sqrt(DEPTH))  # 0.125
NEG = np.float32(-1e9 / SCALE)


def _build_program(mode, use_q_bias, use_k_bias, use_v_bias):
    """mode: 'causal' | 'dense' | 'generic'."""
    nc = bacc.Bacc(
        "TRN2",
        target_bir_lowering=False,
        debug=False,
        enable_asserts=False,
        num_devices=NCORES,
    )

    xq = nc.dram_tensor("xq", [NSB, P, DC, 512], FR, kind="ExternalInput").ap()
    xk = nc.dram_tensor("xk", [NSB, P, DC, 512], FR, kind="ExternalInput").ap()
    xv = nc.dram_tensor("xv", [NSB, P, DC, 512], FR, kind="ExternalInput").ap()
    wq = nc.dram_tensor("wq", [P, DC, CW], FR, kind="ExternalInput").ap()
    wk = nc.dram_tensor("wk", [P, DC, CW], FR, kind="ExternalInput").ap()
    wv = nc.dram_tensor("wv", [P, DC, CW], FR, kind="ExternalInput").ap()
    wo = nc.dram_tensor("wo", [P, CW // P, D], FR, kind="ExternalInput").ap()
    mtri = None
    mneg = None
    if mode == "causal":
        mtri = nc.dram_tensor("mtri", [P, P], F32, kind="ExternalInput").ap()
    elif mode == "generic":
        mneg = nc.dram_tensor("mneg", [S, S], F32, kind="ExternalInput").ap()
    bq = bk = bv = None
    if use_q_bias:
        bq = nc.dram_tensor("bq", [P, CW // P], F32, kind="ExternalInput").ap()
    if use_k_bias:
        bk = nc.dram_tensor("bk", [P, CW // P], F32, kind="ExternalInput").ap()
    if use_v_bias:
        bv = nc.dram_tensor("bv", [P, CW], F32, kind="ExternalInput").ap()
    # [NST, P, 2, 512] has the same memory layout as [S, D] row-major
    out = nc.dram_tensor("out", [NST, P, 2, 512], FR, kind="ExternalOutput").ap()

    with tile.TileContext(nc) as tc, ExitStack() as ctx:
        wpool = ctx.enter_context(tc.tile_pool(name="wpool", bufs=1))
        xpool = ctx.enter_context(tc.tile_pool(name="xpool", bufs=6))
        qkpool = ctx.enter_context(tc.tile_pool(name="qkpool", bufs=1))
        ptpool = ctx.enter_context(tc.tile_pool(name="ptpool", bufs=6))
        smpool = ctx.enter_context(tc.tile_pool(name="smpool", bufs=4))
        outpool = ctx.enter_context(tc.tile_pool(name="outpool", bufs=3))
        mkpool = ctx.enter_context(tc.tile_pool(name="mkpool", bufs=3))
        # PSUM: pf (proj + final, 2x1 bank) + ps (scores pairs, 2x2 banks)
        # + po (attnV accum, 2x1 bank) = 8 banks exactly
        pf = ctx.enter_context(tc.tile_pool(name="pf", bufs=2, space="PSUM"))
        ps = ctx.enter_context(tc.tile_pool(name="ps", bufs=2, space="PSUM"))
        po = ctx.enter_context(tc.tile_pool(name="po", bufs=2, space="PSUM"))

        # --- persistent SBUF tensors (DMAs emitted lazily in the stream) ---
        wq_sb = wpool.tile([P, DC, CW], FR, tag="wq_sb")
        wk_sb = wpool.tile([P, DC, CW], FR, tag="wk_sb")
        wv_sb = wpool.tile([P, DC, CW], FR, tag="wv_sb")
        wo_sb = wpool.tile([P, CW // P, D], FR, tag="wo_sb")
        w_dma = {
            "q": lambda: nc.sync.dma_start(wq_sb[:], wq),
            "k": lambda: nc.sync.dma_start(wk_sb[:], wk),
            "v": lambda: nc.sync.dma_start(wv_sb[:], wv),
            "o": lambda: nc.sync.dma_start(wo_sb[:], wo),
        }
        mtri_sb = None
        if mode == "causal":
            mtri_sb = wpool.tile([P, P], F32, tag="mtri_sb")
        ones_v = wpool.tile([P, HG, 1], F32, tag="ones_v")
        nc.vector.memset(ones_v[:], 1.0)
        # touch Exp once so the ~2.7us activation-table load happens during
        # the startup DMA wait instead of on the first real softmax
        warm_act = wpool.tile([1, 1], F32, tag="warm_act")
        nc.scalar.activation(
            warm_act[:], ones_v[0:1, 0, :],
            mybir.ActivationFunctionType.Exp)
        bq_sb = bk_sb = bv_sb = None
        if use_q_bias:
            bq_sb = wpool.tile([P, CW // P], F32, tag="bq_sb")
            nc.sync.dma_start(bq_sb[:], bq)
        if use_k_bias:
            bk_sb = wpool.tile([P, CW // P], F32, tag="bk_sb")
            nc.sync.dma_start(bk_sb[:], bk)
        if use_v_bias:
            bv_sb = wpool.tile([P, CW], F32, tag="bv_sb")
            nc.sync.dma_start(bv_sb[:], bv)

        # Persistent per-block result tiles (fine-grained deps).
        QT_t = {}  # (cc, sb) -> [128, 512]
        KT_t = {}
        OT_t = {}
        for cc in range(CW // P):
            for sb in range(NSB):
                QT_t[(cc, sb)] = qkpool.tile(
                    [P, 512], FR, name=f"qt_{cc}_{sb}", tag=f"qt_{cc}_{sb}")
                KT_t[(cc, sb)] = qkpool.tile(
                    [P, 512], FR, name=f"kt_{cc}_{sb}", tag=f"kt_{cc}_{sb}")
                OT_t[(cc, sb)] = qkpool.tile(
                    [P, 512], FR, name=f"ot_{cc}_{sb}", tag=f"ot_{cc}_{sb}")
        V_t = {}  # st -> [128, HG, DEPTH+1] (ones col per head)
        for st in range(NST):
            V_t[st] = qkpool.tile(
                [P, HG, DEPTH + 1], FR, name=f"v_{st}", tag=f"v_{st}")

        slabs = {}  # (nm, sl) -> slab tile
        x_of = {"q": xq, "k": xk, "v": xv}

        def load_slab(nm, sl):
            def _c():
                slab = xpool.tile([P, DC, 512], FR, tag="slab",
                                  name=f"sl{nm}_{sl}")
                # two half-slab DMAs: the first 4 dc chunks arrive early so
                # the projection's accumulation chain can start sooner
                h = DC // 2
                nc.sync.dma_start(slab[:, 0:h, :], x_of[nm][sl][:, 0:h, :])
                nc.sync.dma_start(slab[:, h:, :], x_of[nm][sl][:, h:, :])
                slabs[(nm, sl)] = slab
            return _c

        def proj_psum(pool_sel, name):
            # a [P,512] accumulator from either psum pool; the "ps" pool's
            # slot is 2 banks wide, use its first bank
            if pool_sel == "ps":
                t = ps.tile([P, 2, 512], F32, tag="ps", name=name)
                return t[:, 0, :]
            return pf.tile([P, 512], F32, tag="pf", name=name)

        def v_group(sl, sq, pool_sel="pf"):
            def _c():
                st = sl * 4 + sq
                slab = slabs[("v", sl)]
                psum_v = proj_psum(pool_sel, f"pv_{st}")
                for dc in range(DC):
                    nc.tensor.matmul(
                        psum_v[:, :CW],
                        lhsT=slab[:, dc, ts(sq, P)],
                        rhs=wv_sb[:, dc, :],
                        start=(dc == 0),
                        stop=(dc == DC - 1),
                    )
                psrc = psum_v[:, :CW].rearrange("p (h d) -> p h d", h=HG)
                if use_v_bias:
                    nc.vector.tensor_tensor(
                        V_t[st][:, :, 0:DEPTH], psrc,
                        bv_sb.rearrange("p (h d) -> p h d", h=HG),
                        mybir.AluOpType.add,
                    )
                else:
                    nc.vector.tensor_copy(V_t[st][:, :, 0:DEPTH], psrc)
                nc.vector.tensor_copy(
                    V_t[st][:, :, DEPTH : DEPTH + 1], ones_v[:])
            return _c

        def qk_group(nm, cc, sl, pool_sel="pf"):
            w_sb, b_sb, T_t = {
                "q": (wq_sb, bq_sb, QT_t),
                "k": (wk_sb, bk_sb, KT_t),
            }[nm]

            def _c():
                slab = slabs[(nm, sl)]
                psum_q = proj_psum(pool_sel, f"p{nm}_{cc}_{sl}")
                for dc in range(DC):
                    nc.tensor.matmul(
                        psum_q[:],
                        lhsT=w_sb[:, dc, ts(cc, P)],
                        rhs=slab[:, dc, :],
                        start=(dc == 0),
                        stop=(dc == DC - 1),
                    )
                if b_sb is not None:
                    nc.vector.tensor_scalar_add(
                        T_t[(cc, sl)][:], psum_q[:], b_sb[:, cc : cc + 1])
                else:
                    nc.vector.tensor_copy(T_t[(cc, sl)][:], psum_q[:])
            return _c

        def attention_block(i, inject=(), l_on_scalar=()):
            """inject: list of (pos, chunk); chunk emitted when the unit
            counter (cc-major over j) reaches pos. l_on_scalar: cc values
            whose softmax-denominator copies go on ScalarE (tail units,
            where the Vector queue is the bottleneck)."""
            inject = sorted(inject, key=lambda t: t[0])
            inj_idx = 0
            jmax = 4 * i + 4 if mode == "causal" else NST
            jcount = 0
            for cc in range(CW // P):  # head pair (2cc, 2cc+1)
                po0 = po.tile([DEPTH + 1, 512], F32, tag="po",
                              name=f"po0_{i}_{cc}")
                po1 = po.tile([DEPTH + 1, 512], F32, tag="po",
                              name=f"po1_{i}_{cc}")
                pos = (po0, po1)
                for j in range(jmax):
                    while inj_idx < len(inject) and inject[inj_idx][0] <= jcount:
                        inject[inj_idx][1]()
                        inj_idx += 1
                    r = j - 4 * i
                    lo = P * r if (mode == "causal" and r >= 0) else 0
                    psj = ps.tile([P, 2, 512], F32, tag="ps",
                                  name=f"ps_{i}_{cc}_{j}")
                    for hh in range(2):
                        nc.tensor.matmul(
                            psj[:, hh, lo:],
                            lhsT=KT_t[(cc, j // 4)][
                                DEPTH * hh : DEPTH * hh + DEPTH, ts(j % 4, P)],
                            rhs=QT_t[(cc, i)][DEPTH * hh : DEPTH * hh + DEPTH, lo:],
                            start=True,
                            stop=True,
                        )
                    if mode == "causal" and r >= 0:
                        nc.vector.tensor_tensor(
                            psj[:, :, lo : lo + P],
                            psj[:, :, lo : lo + P],
                            mtri_sb[:, None, :].to_broadcast((P, 2, P)),
                            mybir.AluOpType.add,
                        )
                    elif mode == "generic":
                        mk = mkpool.tile([P, 512], F32, tag="mk",
                                         name=f"mk_{i}_{cc}_{j}")
                        nc.sync.dma_start(mk[:], mneg[ts(j, P), ts(i, 512)])
                        nc.vector.tensor_tensor(
                            psj[:], psj[:],
                            mk[:, None, :].to_broadcast((P, 2, 512)),
                            mybir.AluOpType.add,
                        )
                    pt = ptpool.tile([P, 2, 512], FR, tag="pt",
                                     name=f"pt_{i}_{cc}_{j}")
                    nc.scalar.activation(
                        pt[:, :, lo:],
                        psj[:, :, lo:],
                        mybir.ActivationFunctionType.Exp,
                        scale=SCALE,
                    )
                    for hh in range(2):
                        nc.tensor.matmul(
                            pos[hh][:, lo:],
                            lhsT=V_t[j][:, 2 * cc + hh, :],
                            rhs=pt[:, hh, lo:],
                            start=(j == 0),
                            stop=(j == jmax - 1),
                        )
                    jcount += 1
                # normalize both heads: OT[c, q] = outT[c, q] / l[q].
                # l staged to SBUF (custom-DVE recip can't read PSUM); the
                # final scale reads the PSUM accumulator directly.
                l_sb = {}
                rl_sb = {}
                rb = {}
                for hh in range(2):
                    l_sb[hh] = smpool.tile([1, 512], F32, tag="l_sb",
                                           name=f"l_{i}_{cc}_{hh}")
                    if cc in l_on_scalar:
                        nc.scalar.copy(
                            l_sb[hh][:], pos[hh][DEPTH : DEPTH + 1, :])
                    else:
                        nc.vector.tensor_copy(
                            l_sb[hh][:], pos[hh][DEPTH : DEPTH + 1, :])
                for hh in range(2):
                    rl_sb[hh] = smpool.tile([1, 512], F32, tag="rl_sb",
                                            name=f"rl_{i}_{cc}_{hh}")
                    nc.vector.reciprocal_approx_fast(
                        out=rl_sb[hh][:], in_=l_sb[hh][:])
                for hh in range(2):
                    rb[hh] = smpool.tile([DEPTH, 512], F32, tag="rb",
                                         name=f"rb_{i}_{cc}_{hh}")
                    nc.gpsimd.partition_broadcast(rb[hh][:], rl_sb[hh][:])
                for hh in range(2):
                    nc.vector.tensor_tensor(
                        OT_t[(cc, i)][DEPTH * hh : DEPTH * hh + DEPTH, :],
                        pos[hh][0:DEPTH, :],
                        rb[hh][:],
                        mybir.AluOpType.mult,
                    )

            while inj_idx < len(inject):
                inject[inj_idx][1]()
                inj_idx += 1

        def output_chunks(i, split_engines=False, alt_pool=False):
            """8 chunks; each = 2 accumulating fin MMs + copy; one DMA per
            qq once both eh halves are in SBUF."""
            chunks = []
            out_ts = {}

            def fin_group(qq, eh, i=i):
                def _c():
                    qt = 4 * i + qq
                    psum_f = proj_psum(
                        "ps" if (alt_pool and eh == 0) else "pf",
                        f"pfin_{qt}_{eh}")
                    for cc2 in range(CW // P):
                        nc.tensor.matmul(
                            psum_f[:],
                            lhsT=OT_t[(cc2, i)][:, ts(qq, P)],
                            rhs=wo_sb[:, cc2, ts(eh, 512)],
                            start=(cc2 == 0),
                            stop=(cc2 == CW // P - 1),
                        )
                    if eh == 0:
                        out_ts[qq] = outpool.tile(
                            [P, 2, 512], FR, tag="out_t", name=f"ot_{qt}")
                    out_t = out_ts[qq]
                    if split_engines and eh == 1:
                        nc.scalar.copy(out_t[:, eh, :], psum_f[:])
                    else:
                        nc.vector.tensor_copy(out_t[:, eh, :], psum_f[:])
                    if eh == 1:
                        nc.sync.dma_start(out[qt], out_t[:])
                return _c

            for qq in range(4):
                for eh in range(2):
                    chunks.append(fin_group(qq, eh))
            return chunks

        def output_block(i, split_engines=False, alt_pool=False):
            for c in output_chunks(i, split_engines, alt_pool):
                c()

        if mode == "causal":
            # -- pre-phase: minimum projections for attention block 1 --
            w_dma["k"]()
            load_slab("k", 0)()
            w_dma["q"]()
            load_slab("q", 1)()
            w_dma["v"]()
            load_slab("v", 0)()
            nc.sync.dma_start(mtri_sb[:], mtri)
            qk_group("k", 0, 0, "ps")()
            qk_group("k", 1, 0, "pf")()
            qk_group("q", 0, 1, "ps")()
            qk_group("q", 1, 1, "pf")()
            for sq in range(4):
                v_group(0, sq, "ps" if sq % 2 == 0 else "pf")()

            # -- attention phases with positioned PE filler --
            out1 = output_chunks(1)
            out2 = output_chunks(2)
            attention_block(1, inject=[
                (0, load_slab("k", 1)), (0, load_slab("v", 1)),
                (1, qk_group("k", 0, 1)), (2, qk_group("k", 1, 1)),
                (3, v_group(1, 0)), (4, v_group(1, 1)),
                (5, v_group(1, 2)), (6, v_group(1, 3)),
                (7, load_slab("q", 2)), (8, w_dma["o"]),
                (9, load_slab("k", 2)), (11, load_slab("v", 2)),
                (13, qk_group("q", 0, 2)),
            ])
            attention_block(2, inject=[
                (0, qk_group("q", 1, 2)),
                (1, qk_group("k", 0, 2)), (3, qk_group("k", 1, 2)),
                (4, v_group(2, 0)), (5, v_group(2, 1)),
                (6, v_group(2, 2)), (7, v_group(2, 3)),
                (9, load_slab("q", 3)), (10, qk_group("q", 0, 3)),
                (12, qk_group("q", 1, 3)),
                (14, load_slab("k", 3)), (19, load_slab("v", 3)),
            ])
            attention_block(3, inject=[
                (0, load_slab("q", 0)),
                (1, qk_group("k", 0, 3)), (3, qk_group("k", 1, 3)),
                (2, v_group(3, 0)), (4, v_group(3, 1)),
                (6, v_group(3, 2)), (8, v_group(3, 3)),
                (10, qk_group("q", 0, 0)), (12, qk_group("q", 1, 0)),
                (14, out1[0]), (16, out1[1]), (18, out1[2]),
                (20, out1[3]), (22, out1[4]), (24, out1[5]),
                (26, out1[6]), (28, out1[7]),
            ], l_on_scalar=(1,))
            # out2's fins pinned at the att3->att0 po-ring stall (position
            # 0): the PE blocks there on att3's last normalize, and filler
            # behind the blocked attnV in the queue cannot run. out3's fins
            # (ready once att3's normalize lands) cover att0's second half
            # and block 0's own normalize chains.
            out3 = output_chunks(3, split_engines=True, alt_pool=True)
            attention_block(0, inject=[
                (0, out2[0]), (0, out2[1]), (0, out2[2]), (0, out2[3]),
                (0, out2[4]), (0, out2[5]), (0, out2[6]), (0, out2[7]),
                (4, out3[0]), (4, out3[1]), (5, out3[2]), (5, out3[3]),
                (6, out3[4]), (6, out3[5]), (7, out3[6]), (7, out3[7]),
            ], l_on_scalar=(0, 1))
            # tail: only block 0's own output projection remains
            output_block(0, split_engines=True, alt_pool=True)
        else:
            # dense/generic need all KT/V before any attention block
            w_dma["o"]()
            for nm in ("v", "q", "k"):
                w_dma[nm]()
            for sl in range(NSB):
                load_slab("v", sl)()
                for sq in range(4):
                    v_group(sl, sq)()
                load_slab("q", sl)()
                for cc in range(CW // P):
                    qk_group("q", cc, sl)()
                load_slab("k", sl)()
                for cc in range(CW // P):
                    qk_group("k", cc, sl)()
            for i in range(NSB):
                attention_block(i)
                output_block(i)

    nc.compile()
    return nc


_PROG_CACHE = {}


def _get_program(mode, use_q_bias, use_k_bias, use_v_bias):
    key = (mode, use_q_bias, use_k_bias, use_v_bias)
    if key not in _PROG_CACHE:
        _PROG_CACHE[key] = _build_program(mode, use_q_bias, use_k_bias, use_v_bias)
    return _PROG_CACHE[key]


import ml_dtypes


def _pretile(x2d):
    # [S, D] -> [NSB, P, DC, 512]: arr[sl, p, dc, s] = x2d[sl*512+s, dc*128+p]
    return np.ascontiguousarray(
        x2d.reshape(NSB, 512, DC, P).transpose(0, 3, 2, 1)
    ).astype(ml_dtypes.bfloat16)


def _pretile_w(w):
    # [D, CW] -> [P, DC, CW]
    return np.ascontiguousarray(
        w.reshape(DC, P, CW).transpose(1, 0, 2)).astype(ml_dtypes.bfloat16)


def kernel(**inputs):
    query = np.asarray(inputs["query"], np.float32)
    key = np.asarray(inputs["key"], np.float32)
    value = np.asarray(inputs["value"], np.float32)
    mask = np.asarray(inputs["mask"], np.float32).reshape(S, S)
    wq = np.asarray(inputs["wq"], np.float32)
    wk = np.asarray(inputs["wk"], np.float32)
    wv = np.asarray(inputs["wv"], np.float32)
    wo = np.asarray(inputs["wo"], np.float32)
    bq = np.asarray(inputs["bq"], np.float32)
    bk = np.asarray(inputs["bk"], np.float32)
    bv = np.asarray(inputs["bv"], np.float32)
    bo = np.asarray(inputs["bo"], np.float32)

    if not mask.any():
        mode = "dense"
    elif np.array_equal(mask, np.triu(np.ones((S, S), np.float32), 1)):
        mode = "causal"
    else:
        mode = "generic"
    use_q_bias = bool(bq.any())
    use_k_bias = bool(bk.any())
    use_v_bias = bool(bv.any())

    nc = _get_program(mode, use_q_bias, use_k_bias, use_v_bias)

    in_maps = []
    for core in range(NCORES):
        b, g = core // GROUPS, core % GROUPS
        cs = slice(g * CW, (g + 1) * CW)
        m = {
            "xq": _pretile(query[b]),
            "xk": _pretile(key[b]),
            "xv": _pretile(value[b]),
            "wq": _pretile_w(wq[:, cs]),
            "wk": _pretile_w(wk[:, cs]),
            "wv": _pretile_w(wv[:, cs]),
            "wo": np.ascontiguousarray(
                wo[cs, :].reshape(CW // P, P, D).transpose(1, 0, 2)
            ).astype(ml_dtypes.bfloat16),
        }
        if mode == "causal":
            m["mtri"] = np.where(
                np.triu(np.ones((P, P), bool), 0), np.float32(0), NEG
            ).astype(np.float32)
        elif mode == "generic":
            m["mneg"] = np.ascontiguousarray(mask.T) * NEG
        if use_q_bias:
            m["bq"] = np.ascontiguousarray(bq[cs].reshape(CW // P, P).T)
        if use_k_bias:
            m["bk"] = np.ascontiguousarray(bk[cs].reshape(CW // P, P).T)
        if use_v_bias:
            m["bv"] = np.ascontiguousarray(np.tile(bv[cs], (P, 1)))
        in_maps.append(m)

    res = bass_utils.run_bass_kernel_spmd(
        nc, in_maps, core_ids=list(range(NCORES)), trace=False
    )
    outs = [np.asarray(r["out"], np.float32).reshape(S, D) for r in res.results]
    full = np.empty((B, S, D), np.float32)
    for b in range(B):
        full[b] = outs[GROUPS * b]
        for g in range(1, GROUPS):
            full[b] += outs[GROUPS * b + g]
        full[b] += bo
    return full


# revision 32
# speedup vs baseline: 1.0526x; 1.0121x over previous
"""Multi-head attention (B=2, S=2048, D=1024, H=16) on 8 Trainium2 cores.

Sharding: core = (batch b, head-group g): 2 batches x 4 groups of 4 heads.
Each core computes Q/K/V projections for its 256 model columns, causal
attention for its 4 heads, and a partial output projection through its
256 rows of Wo. Host sums the 4 partials per batch (the "all-reduce").

Device-side layout strategy (per core):
  - Host passes query/key/value pre-tiled+transposed: [NSB, 128, 8, 512]
    (contiguous 8KB-per-partition DMA runs, one descriptor set per slab).
  - QT/KT [c=256, s] produced directly with W stationary (full-speed MMs).
  - V [s, c] produced with xT stationary, padded with a ones column per
    head so the attnV matmul also yields the softmax denominator l.
  - Scores computed transposed: ST[k, q], one psum tile per (head-pair, j)
    holding both heads (row-disjoint matmuls overlap in the PE array);
    additive causal mask on the diagonal 128-blocks; q range trimmed to
    [128*r, 512) on diagonal blocks; exp on ScalarE with fused 1/sqrt(64)
    scale (max-subtraction skipped: scores bounded).
  - attnV: outT[d(+l), q] = V_aug^T @ PT, accumulated over k blocks in
    PSUM; columns below the causal diagonal are skipped entirely.
  - Normalize with reciprocal_approx_fast + GpSimd partition_broadcast;
    the final scale reads PSUM directly (no staging copy).
  - Output projection: lhsT = OT chunks, rhs = Wo -> partial out [s, e],
    written back as bf16 (tolerance allows it; halves the write traffic).

Global schedule (causal): attention q-blocks processed in order 1,2,3,0.
The exp on ScalarE is the throughput limit inside an attention block
(1.15us per 128-k-block vs 0.64us of PE work), so projection and
output-projection chunks are pinned at explicit positions inside each
attention block to keep the PE busy during exp gaps, weighted so each
phase's PE filler matches its ScalarE time. Processing block 0 (all
diagonal, tiny exp cost) last keeps the tail short and the PE warm.
All matmuls use bf16 operands (full PE speed).
"""

import os
import numpy as np
from contextlib import ExitStack

import concourse.bass as bass
import concourse.tile as tile
from concourse import bacc, mybir
from concourse import bass_utils
from concourse.bass import ts

B, S, D, H = 2, 2048, 1024, 16
DEPTH = D // H            # 64
NCORES = 8
GROUPS = 4                # head-groups per batch
HG = H // GROUPS          # 4 heads per core
CW = HG * DEPTH           # 256 local columns
P = 128
DC = D // P               # 8 contraction chunks
NST = S // P              # 16 seq tiles of 128
NSB = S // 512            # 4 seq blocks of 512
F32 = mybir.dt.float32
FR = mybir.dt.bfloat16
SCALE = 1.0 / float(np.sqrt(DEPTH))  # 0.125
NEG = np.float32(-1e9 / SCALE)


def _build_program(mode, use_q_bias, use_k_bias, use_v_bias):
    """mode: 'causal' | 'dense' | 'generic'."""
    nc = bacc.Bacc(
        "TRN2",
        target_bir_lowering=False,
        debug=False,
        enable_asserts=False,
        num_devices=NCORES,
    )

    xq = nc.dram_tensor("xq", [NSB, P, DC, 512], FR, kind="ExternalInput").ap()
    xk = nc.dram_tensor("xk", [NSB, P, DC, 512], FR, kind="ExternalInput").ap()
    xv = nc.dram_tensor("xv", [NSB, P, DC, 512], FR, kind="ExternalInput").ap()
    wq = nc.dram_tensor("wq", [P, DC, CW], FR, kind="ExternalInput").ap()
    wk = nc.dram_tensor("wk", [P, DC, CW], FR, kind="ExternalInput").ap()
    wv = nc.dram_tensor("wv", [P, DC, CW], FR, kind="ExternalInput").ap()
    wo = nc.dram_tensor("wo", [P, CW // P, D], FR, kind="ExternalInput").ap()
    mtri = None
    mneg = None
    if mode == "causal":
        # 0/1 keep-mask (1 where k <= q): applied to exp's OUTPUT, so the
        # scalar engine never waits on a vector-engine mask add
        mtri = nc.dram_tensor("mtri01", [P, P], F32, kind="ExternalInput").ap()
    elif mode == "generic":
        mneg = nc.dram_tensor("mneg", [S, S], F32, kind="ExternalInput").ap()
    bq = bk = bv = None
    if use_q_bias:
        bq = nc.dram_tensor("bq", [P, CW // P], F32, kind="ExternalInput").ap()
    if use_k_bias:
        bk = nc.dram_tensor("bk", [P, CW // P], F32, kind="ExternalInput").ap()
    if use_v_bias:
        bv = nc.dram_tensor("bv", [P, CW], F32, kind="ExternalInput").ap()
    # [NST, P, 2, 512] has the same memory layout as [S, D] row-major
    out = nc.dram_tensor("out", [NST, P, 2, 512], FR, kind="ExternalOutput").ap()

    with tile.TileContext(nc) as tc, ExitStack() as ctx:
        wpool = ctx.enter_context(tc.tile_pool(name="wpool", bufs=1))
        xpool = ctx.enter_context(tc.tile_pool(name="xpool", bufs=6))
        qkpool = ctx.enter_context(tc.tile_pool(name="qkpool", bufs=1))
        ptpool = ctx.enter_context(tc.tile_pool(name="ptpool", bufs=6))
        smpool = ctx.enter_context(tc.tile_pool(name="smpool", bufs=4))
        outpool = ctx.enter_context(tc.tile_pool(name="outpool", bufs=3))
        mkpool = ctx.enter_context(tc.tile_pool(name="mkpool", bufs=3))
        # PSUM: pf (proj + final, 2x1 bank) + ps (scores pairs, 2x2 banks)
        # + po (attnV accum, 2x1 bank) = 8 banks exactly
        pf = ctx.enter_context(tc.tile_pool(name="pf", bufs=2, space="PSUM"))
        ps = ctx.enter_context(tc.tile_pool(name="ps", bufs=2, space="PSUM"))
        po = ctx.enter_context(tc.tile_pool(name="po", bufs=2, space="PSUM"))

        # --- persistent SBUF tensors (DMAs emitted lazily in the stream) ---
        wq_sb = wpool.tile([P, DC, CW], FR, tag="wq_sb")
        wk_sb = wpool.tile([P, DC, CW], FR, tag="wk_sb")
        wv_sb = wpool.tile([P, DC, CW], FR, tag="wv_sb")
        wo_sb = wpool.tile([P, CW // P, D], FR, tag="wo_sb")
        w_dma = {
            "q": lambda: nc.sync.dma_start(wq_sb[:], wq),
            "k": lambda: nc.sync.dma_start(wk_sb[:], wk),
            "v": lambda: nc.sync.dma_start(wv_sb[:], wv),
            "o": lambda: nc.sync.dma_start(wo_sb[:], wo),
        }
        mtri_sb = None
        if mode == "causal":
            mtri_sb = wpool.tile([P, P], F32, tag="mtri_sb")
        ones_v = wpool.tile([P, HG, 1], F32, tag="ones_v")
        nc.vector.memset(ones_v[:], 1.0)
        # touch Exp once so the ~2.7us activation-table load happens during
        # the startup DMA wait instead of on the first real softmax
        warm_act = wpool.tile([1, 1], F32, tag="warm_act")
        nc.scalar.activation(
            warm_act[:], ones_v[0:1, 0, :],
            mybir.ActivationFunctionType.Exp)
        bq_sb = bk_sb = bv_sb = None
        if use_q_bias:
            bq_sb = wpool.tile([P, CW // P], F32, tag="bq_sb")
            nc.sync.dma_start(bq_sb[:], bq)
        if use_k_bias:
            bk_sb = wpool.tile([P, CW // P], F32, tag="bk_sb")
            nc.sync.dma_start(bk_sb[:], bk)
        if use_v_bias:
            bv_sb = wpool.tile([P, CW], F32, tag="bv_sb")
            nc.sync.dma_start(bv_sb[:], bv)

        # Persistent per-block result tiles (fine-grained deps).
        QT_t = {}  # (cc, sb) -> [128, 512]
        KT_t = {}
        OT_t = {}
        for cc in range(CW // P):
            for sb in range(NSB):
                QT_t[(cc, sb)] = qkpool.tile(
                    [P, 512], FR, name=f"qt_{cc}_{sb}", tag=f"qt_{cc}_{sb}")
                KT_t[(cc, sb)] = qkpool.tile(
                    [P, 512], FR, name=f"kt_{cc}_{sb}", tag=f"kt_{cc}_{sb}")
                OT_t[(cc, sb)] = qkpool.tile(
                    [P, 512], FR, name=f"ot_{cc}_{sb}", tag=f"ot_{cc}_{sb}")
        V_t = {}  # st -> [128, HG, DEPTH+1] (ones col per head)
        for st in range(NST):
            V_t[st] = qkpool.tile(
                [P, HG, DEPTH + 1], FR, name=f"v_{st}", tag=f"v_{st}")
            # the ones column never changes: write it once at setup
            nc.vector.tensor_copy(
                V_t[st][:, :, DEPTH : DEPTH + 1], ones_v[:])

        slabs = {}  # (nm, sl) -> slab tile
        x_of = {"q": xq, "k": xk, "v": xv}

        def load_slab(nm, sl):
            def _c():
                slab = xpool.tile([P, DC, 512], FR, tag="slab",
                                  name=f"sl{nm}_{sl}")
                # two half-slab DMAs: the first 4 dc chunks arrive early so
                # the projection's accumulation chain can start sooner
                h = DC // 2
                nc.sync.dma_start(slab[:, 0:h, :], x_of[nm][sl][:, 0:h, :])
                nc.sync.dma_start(slab[:, h:, :], x_of[nm][sl][:, h:, :])
                slabs[(nm, sl)] = slab
            return _c

        def proj_psum(pool_sel, name):
            # a [P,512] accumulator from either psum pool; the "ps" pool's
            # slot is 2 banks wide, use its first bank
            if pool_sel == "ps":
                t = ps.tile([P, 2, 512], F32, tag="ps", name=name)
                return t[:, 0, :]
            return pf.tile([P, 512], F32, tag="pf", name=name)

        def v_group(sl, sq, pool_sel="pf"):
            def _c():
                st = sl * 4 + sq
                slab = slabs[("v", sl)]
                psum_v = proj_psum(pool_sel, f"pv_{st}")
                for dc in range(DC):
                    nc.tensor.matmul(
                        psum_v[:, :CW],
                        lhsT=slab[:, dc, ts(sq, P)],
                        rhs=wv_sb[:, dc, :],
                        start=(dc == 0),
                        stop=(dc == DC - 1),
                    )
                psrc = psum_v[:, :CW].rearrange("p (h d) -> p h d", h=HG)
                if use_v_bias:
                    nc.vector.tensor_tensor(
                        V_t[st][:, :, 0:DEPTH], psrc,
                        bv_sb.rearrange("p (h d) -> p h d", h=HG),
                        mybir.AluOpType.add,
                    )
                else:
                    nc.vector.tensor_copy(V_t[st][:, :, 0:DEPTH], psrc)
            return _c

        def qk_group(nm, cc, sl, pool_sel="pf"):
            w_sb, b_sb, T_t = {
                "q": (wq_sb, bq_sb, QT_t),
                "k": (wk_sb, bk_sb, KT_t),
            }[nm]

            def _c():
                slab = slabs[(nm, sl)]
                psum_q = proj_psum(pool_sel, f"p{nm}_{cc}_{sl}")
                for dc in range(DC):
                    nc.tensor.matmul(
                        psum_q[:],
                        lhsT=w_sb[:, dc, ts(cc, P)],
                        rhs=slab[:, dc, :],
                        start=(dc == 0),
                        stop=(dc == DC - 1),
                    )
                if b_sb is not None:
                    nc.vector.tensor_scalar_add(
                        T_t[(cc, sl)][:], psum_q[:], b_sb[:, cc : cc + 1])
                else:
                    nc.vector.tensor_copy(T_t[(cc, sl)][:], psum_q[:])
            return _c

        def attention_block(i, inject=(), l_on_scalar=()):
            """inject: list of (pos, chunk); chunk emitted when the unit
            counter (cc-major over j) reaches pos. l_on_scalar: cc values
            whose softmax-denominator copies go on ScalarE (tail units,
            where the Vector queue is the bottleneck)."""
            inject = sorted(inject, key=lambda t: t[0])
            inj_idx = 0
            jmax = 4 * i + 4 if mode == "causal" else NST
            jcount = 0
            for cc in range(CW // P):  # head pair (2cc, 2cc+1)
                po0 = po.tile([DEPTH + 1, 512], F32, tag="po",
                              name=f"po0_{i}_{cc}")
                po1 = po.tile([DEPTH + 1, 512], F32, tag="po",
                              name=f"po1_{i}_{cc}")
                pos = (po0, po1)
                for j in range(jmax):
                    while inj_idx < len(inject) and inject[inj_idx][0] <= jcount:
                        inject[inj_idx][1]()
                        inj_idx += 1
                    r = j - 4 * i
                    lo = P * r if (mode == "causal" and r >= 0) else 0
                    psj = ps.tile([P, 2, 512], F32, tag="ps",
                                  name=f"ps_{i}_{cc}_{j}")
                    for hh in range(2):
                        nc.tensor.matmul(
                            psj[:, hh, lo:],
                            lhsT=KT_t[(cc, j // 4)][
                                DEPTH * hh : DEPTH * hh + DEPTH, ts(j % 4, P)],
                            rhs=QT_t[(cc, i)][DEPTH * hh : DEPTH * hh + DEPTH, lo:],
                            start=True,
                            stop=True,
                        )
                    if mode == "generic":
                        mk = mkpool.tile([P, 512], F32, tag="mk",
                                         name=f"mk_{i}_{cc}_{j}")
                        nc.sync.dma_start(mk[:], mneg[ts(j, P), ts(i, 512)])
                        nc.vector.tensor_tensor(
                            psj[:], psj[:],
                            mk[:, None, :].to_broadcast((P, 2, 512)),
                            mybir.AluOpType.add,
                        )
                    pt = ptpool.tile([P, 2, 512], FR, tag="pt",
                                     name=f"pt_{i}_{cc}_{j}")
                    nc.scalar.activation(
                        pt[:, :, lo:],
                        psj[:, :, lo:],
                        mybir.ActivationFunctionType.Exp,
                        scale=SCALE,
                    )
                    if mode == "causal" and r >= 0:
                        # zero the below-diagonal strip of exp's output
                        nc.vector.tensor_tensor(
                            pt[:, :, lo : lo + P],
                            pt[:, :, lo : lo + P],
                            mtri_sb[:, None, :].to_broadcast((P, 2, P)),
                            mybir.AluOpType.mult,
                        )
                    for hh in range(2):
                        nc.tensor.matmul(
                            pos[hh][:, lo:],
                            lhsT=V_t[j][:, 2 * cc + hh, :],
                            rhs=pt[:, hh, lo:],
                            start=(j == 0),
                            stop=(j == jmax - 1),
                        )
                    jcount += 1
                # normalize both heads: OT[c, q] = outT[c, q] / l[q].
                # l staged to SBUF (custom-DVE recip can't read PSUM); the
                # final scale reads the PSUM accumulator directly.
                l_sb = {}
                rl_sb = {}
                rb = {}
                for hh in range(2):
                    l_sb[hh] = smpool.tile([1, 512], F32, tag="l_sb",
                                           name=f"l_{i}_{cc}_{hh}")
                    if cc in l_on_scalar:
                        nc.scalar.copy(
                            l_sb[hh][:], pos[hh][DEPTH : DEPTH + 1, :])
                    else:
                        nc.vector.tensor_copy(
                            l_sb[hh][:], pos[hh][DEPTH : DEPTH + 1, :])
                for hh in range(2):
                    rl_sb[hh] = smpool.tile([1, 512], F32, tag="rl_sb",
                                            name=f"rl_{i}_{cc}_{hh}")
                    nc.vector.reciprocal_approx_fast(
                        out=rl_sb[hh][:], in_=l_sb[hh][:])
                for hh in range(2):
                    rb[hh] = smpool.tile([DEPTH, 512], F32, tag="rb",
                                         name=f"rb_{i}_{cc}_{hh}")
                    nc.gpsimd.partition_broadcast(rb[hh][:], rl_sb[hh][:])
                for hh in range(2):
                    nc.vector.tensor_tensor(
                        OT_t[(cc, i)][DEPTH * hh : DEPTH * hh + DEPTH, :],
                        pos[hh][0:DEPTH, :],
                        rb[hh][:],
                        mybir.AluOpType.mult,
                    )

            while inj_idx < len(inject):
                inject[inj_idx][1]()
                inj_idx += 1

        def output_chunks(i, split_engines=False, alt_pool=False):
            """8 chunks; each = 2 accumulating fin MMs + copy; one DMA per
            qq once both eh halves are in SBUF."""
            chunks = []
            out_ts = {}

            def fin_group(qq, eh, i=i):
                def _c():
                    qt = 4 * i + qq
                    psum_f = proj_psum(
                        "ps" if (alt_pool and eh == 0) else "pf",
                        f"pfin_{qt}_{eh}")
                    for cc2 in range(CW // P):
                        nc.tensor.matmul(
                            psum_f[:],
                            lhsT=OT_t[(cc2, i)][:, ts(qq, P)],
                            rhs=wo_sb[:, cc2, ts(eh, 512)],
                            start=(cc2 == 0),
                            stop=(cc2 == CW // P - 1),
                        )
                    if eh == 0:
                        out_ts[qq] = outpool.tile(
                            [P, 2, 512], FR, tag="out_t", name=f"ot_{qt}")
                    out_t = out_ts[qq]
                    if split_engines and eh == 1:
                        nc.scalar.copy(out_t[:, eh, :], psum_f[:])
                    else:
                        nc.vector.tensor_copy(out_t[:, eh, :], psum_f[:])
                    if eh == 1:
                        nc.sync.dma_start(out[qt], out_t[:])
                return _c

            for qq in range(4):
                for eh in range(2):
                    chunks.append(fin_group(qq, eh))
            return chunks

        def output_block(i, split_engines=False, alt_pool=False):
            for c in output_chunks(i, split_engines, alt_pool):
                c()

        if mode == "causal":
            # -- pre-phase: minimum projections for attention block 1 --
            w_dma["k"]()
            load_slab("k", 0)()
            w_dma["q"]()
            load_slab("q", 1)()
            w_dma["v"]()
            load_slab("v", 0)()
            nc.sync.dma_start(mtri_sb[:], mtri)
            qk_group("k", 0, 0, "ps")()
            qk_group("k", 1, 0, "pf")()
            qk_group("q", 0, 1, "ps")()
            qk_group("q", 1, 1, "pf")()
            for sq in range(4):
                v_group(0, sq, "ps" if sq % 2 == 0 else "pf")()

            # -- attention phases with positioned PE filler --
            out1 = output_chunks(1)
            out2 = output_chunks(2)
            attention_block(1, inject=[
                (0, load_slab("k", 1)), (0, load_slab("v", 1)),
                (1, qk_group("k", 0, 1)), (2, qk_group("k", 1, 1)),
                (3, v_group(1, 0)), (4, v_group(1, 1)),
                (5, v_group(1, 2)), (6, v_group(1, 3)),
                (7, load_slab("q", 2)), (8, w_dma["o"]),
                (9, load_slab("k", 2)), (11, load_slab("v", 2)),
                (13, qk_group("q", 0, 2)),
            ])
            attention_block(2, inject=[
                (0, qk_group("q", 1, 2)),
                (1, qk_group("k", 0, 2)), (3, qk_group("k", 1, 2)),
                (4, v_group(2, 0)), (5, v_group(2, 1)),
                (6, v_group(2, 2)), (7, v_group(2, 3)),
                (9, load_slab("q", 3)), (10, qk_group("q", 0, 3)),
                (12, qk_group("q", 1, 3)),
                (14, load_slab("k", 3)), (19, load_slab("v", 3)),
            ])
            attention_block(3, inject=[
                (0, load_slab("q", 0)),
                (1, qk_group("k", 0, 3)), (3, qk_group("k", 1, 3)),
                (2, v_group(3, 0)), (4, v_group(3, 1)),
                (6, v_group(3, 2)), (8, v_group(3, 3)),
                (10, qk_group("q", 0, 0)), (12, qk_group("q", 1, 0)),
                (14, out1[0]), (16, out1[1]), (18, out1[2]),
                (20, out1[3]), (22, out1[4]), (24, out1[5]),
                (26, out1[6]), (28, out1[7]),
            ], l_on_scalar=(1,))
            # out2's fins pinned at the att3->att0 po-ring stall (position
            # 0): the PE blocks there on att3's last normalize, and filler
            # behind the blocked attnV in the queue cannot run. out3's fins
            # (ready once att3's normalize lands) cover att0's second half
            # and block 0's own normalize chains.
            out3 = output_chunks(3, split_engines=True, alt_pool=True)
            attention_block(0, inject=[
                (0, out2[0]), (0, out2[1]), (0, out2[2]), (0, out2[3]),
                (0, out2[4]), (0, out2[5]), (0, out2[6]), (0, out2[7]),
                (4, out3[0]), (4, out3[1]), (5, out3[2]), (5, out3[3]),
                (6, out3[4]), (6, out3[5]), (7, out3[6]), (7, out3[7]),
            ], l_on_scalar=(0, 1))
            # tail: only block 0's own output projection remains
            output_block(0, split_engines=True, alt_pool=True)
        else:
            # dense/generic need all KT/V before any attention block
            w_dma["o"]()
            for nm in ("v", "q", "k"):
                w_dma[nm]()
            for sl in range(NSB):
                load_slab("v", sl)()
                for sq in range(4):
                    v_group(sl, sq)()
                load_slab("q", sl)()
                for cc in range(CW // P):
                    qk_group("q", cc, sl)()
                load_slab("k", sl)()
                for cc in range(CW // P):
                    qk_group("k", cc, sl)()
            for i in range(NSB):
                attention_block(i)
                output_block(i)

    nc.compile()
    return nc


_PROG_CACHE = {}


def _get_program(mode, use_q_bias, use_k_bias, use_v_bias):
    key = (mode, use_q_bias, use_k_bias, use_v_bias)
    if key not in _PROG_CACHE:
        _PROG_CACHE[key] = _build_program(mode, use_q_bias, use_k_bias, use_v_bias)
    return _PROG_CACHE[key]


import ml_dtypes


def _pretile(x2d):
    # [S, D] -> [NSB, P, DC, 512]: arr[sl, p, dc, s] = x2d[sl*512+s, dc*128+p]
    return np.ascontiguousarray(
        x2d.reshape(NSB, 512, DC, P).transpose(0, 3, 2, 1)
    ).astype(ml_dtypes.bfloat16)


def _pretile_w(w):
    # [D, CW] -> [P, DC, CW]
    return np.ascontiguousarray(
        w.reshape(DC, P, CW).transpose(1, 0, 2)).astype(ml_dtypes.bfloat16)


def kernel(**inputs):
    query = np.asarray(inputs["query"], np.float32)
    key = np.asarray(inputs["key"], np.float32)
    value = np.asarray(inputs["value"], np.float32)
    mask = np.asarray(inputs["mask"], np.float32).reshape(S, S)
    wq = np.asarray(inputs["wq"], np.float32)
    wk = np.asarray(inputs["wk"], np.float32)
    wv = np.asarray(inputs["wv"], np.float32)
    wo = np.asarray(inputs["wo"], np.float32)
    bq = np.asarray(inputs["bq"], np.float32)
    bk = np.asarray(inputs["bk"], np.float32)
    bv = np.asarray(inputs["bv"], np.float32)
    bo = np.asarray(inputs["bo"], np.float32)

    if not mask.any():
        mode = "dense"
    elif np.array_equal(mask, np.triu(np.ones((S, S), np.float32), 1)):
        mode = "causal"
    else:
        mode = "generic"
    use_q_bias = bool(bq.any())
    use_k_bias = bool(bk.any())
    use_v_bias = bool(bv.any())

    nc = _get_program(mode, use_q_bias, use_k_bias, use_v_bias)

    in_maps = []
    for core in range(NCORES):
        b, g = core // GROUPS, core % GROUPS
        cs = slice(g * CW, (g + 1) * CW)
        m = {
            "xq": _pretile(query[b]),
            "xk": _pretile(key[b]),
            "xv": _pretile(value[b]),
            "wq": _pretile_w(wq[:, cs]),
            "wk": _pretile_w(wk[:, cs]),
            "wv": _pretile_w(wv[:, cs]),
            "wo": np.ascontiguousarray(
                wo[cs, :].reshape(CW // P, P, D).transpose(1, 0, 2)
            ).astype(ml_dtypes.bfloat16),
        }
        if mode == "causal":
            m["mtri01"] = np.triu(np.ones((P, P), np.float32), 0)
        elif mode == "generic":
            m["mneg"] = np.ascontiguousarray(mask.T) * NEG
        if use_q_bias:
            m["bq"] = np.ascontiguousarray(bq[cs].reshape(CW // P, P).T)
        if use_k_bias:
            m["bk"] = np.ascontiguousarray(bk[cs].reshape(CW // P, P).T)
        if use_v_bias:
            m["bv"] = np.ascontiguousarray(np.tile(bv[cs], (P, 1)))
        in_maps.append(m)

    res = bass_utils.run_bass_kernel_spmd(
        nc, in_maps, core_ids=list(range(NCORES)), trace=False
    )
    outs = [np.asarray(r["out"], np.float32).reshape(S, D) for r in res.results]
    full = np.empty((B, S, D), np.float32)
    for b in range(B):
        full[b] = outs[GROUPS * b]
        for g in range(1, GROUPS):
            full[b] += outs[GROUPS * b + g]
        full[b] += bo
    return full


# revision 35
# speedup vs baseline: 1.0631x; 1.0100x over previous
"""Multi-head attention (B=2, S=2048, D=1024, H=16) on 8 Trainium2 cores.

Sharding: core = (batch b, head-group g): 2 batches x 4 groups of 4 heads.
Each core computes Q/K/V projections for its 256 model columns, causal
attention for its 4 heads, and a partial output projection through its
256 rows of Wo. Host sums the 4 partials per batch (the "all-reduce").

Device-side layout strategy (per core):
  - Host passes query/key/value pre-tiled+transposed: [NSB, 128, 8, 512]
    (contiguous 8KB-per-partition DMA runs, one descriptor set per slab).
  - QT/KT [c=256, s] produced directly with W stationary (full-speed MMs).
  - V [s, c] produced with xT stationary, padded with a ones column per
    head so the attnV matmul also yields the softmax denominator l.
  - Scores computed transposed: ST[k, q], one psum tile per (head-pair, j)
    holding both heads (row-disjoint matmuls overlap in the PE array);
    additive causal mask on the diagonal 128-blocks; q range trimmed to
    [128*r, 512) on diagonal blocks; exp on ScalarE with fused 1/sqrt(64)
    scale (max-subtraction skipped: scores bounded).
  - attnV: outT[d(+l), q] = V_aug^T @ PT, accumulated over k blocks in
    PSUM; columns below the causal diagonal are skipped entirely.
  - Normalize with reciprocal_approx_fast + GpSimd partition_broadcast;
    the final scale reads PSUM directly (no staging copy).
  - Output projection: lhsT = OT chunks, rhs = Wo -> partial out [s, e],
    written back as bf16 (tolerance allows it; halves the write traffic).

Global schedule (causal): attention q-blocks processed in order 1,2,3,0.
The exp on ScalarE is the throughput limit inside an attention block
(1.15us per 128-k-block vs 0.64us of PE work), so projection and
output-projection chunks are pinned at explicit positions inside each
attention block to keep the PE busy during exp gaps, weighted so each
phase's PE filler matches its ScalarE time. Processing block 0 (all
diagonal, tiny exp cost) last keeps the tail short and the PE warm.
All matmuls use bf16 operands (full PE speed).
"""

import os
import numpy as np
from contextlib import ExitStack

import concourse.bass as bass
import concourse.tile as tile
from concourse import bacc, mybir
from concourse import bass_utils
from concourse.bass import ts

B, S, D, H = 2, 2048, 1024, 16
DEPTH = D // H            # 64
NCORES = 8
GROUPS = 4                # head-groups per batch
HG = H // GROUPS          # 4 heads per core
CW = HG * DEPTH           # 256 local columns
P = 128
DC = D // P               # 8 contraction chunks
NST = S // P              # 16 seq tiles of 128
NSB = S // 512            # 4 seq blocks of 512
F32 = mybir.dt.float32
FR = mybir.dt.bfloat16
SCALE = 1.0 / float(np.sqrt(DEPTH))  # 0.125
NEG = np.float32(-1e9 / SCALE)


def _build_program(mode, use_q_bias, use_k_bias, use_v_bias):
    """mode: 'causal' | 'dense' | 'generic'."""
    nc = bacc.Bacc(
        "TRN2",
        target_bir_lowering=False,
        debug=False,
        enable_asserts=False,
        num_devices=NCORES,
    )

    xq = nc.dram_tensor("xq", [NSB, P, DC, 512], FR, kind="ExternalInput").ap()
    xk = nc.dram_tensor("xk", [NSB, P, DC, 512], FR, kind="ExternalInput").ap()
    xv = nc.dram_tensor("xv", [NSB, P, DC, 512], FR, kind="ExternalInput").ap()
    wq = nc.dram_tensor("wq", [P, DC, CW], FR, kind="ExternalInput").ap()
    wk = nc.dram_tensor("wk", [P, DC, CW], FR, kind="ExternalInput").ap()
    wv = nc.dram_tensor("wv", [P, DC, CW], FR, kind="ExternalInput").ap()
    wo = nc.dram_tensor("wo", [P, CW // P, D], FR, kind="ExternalInput").ap()
    mtri = None
    mneg = None
    if mode == "causal":
        # 0/1 keep-mask (1 where k <= q): applied to exp's OUTPUT, so the
        # scalar engine never waits on a vector-engine mask add
        mtri = nc.dram_tensor("mtri01", [P, P], F32, kind="ExternalInput").ap()
    elif mode == "generic":
        mneg = nc.dram_tensor("mneg", [S, S], F32, kind="ExternalInput").ap()
    bq = bk = bv = None
    if use_q_bias:
        bq = nc.dram_tensor("bq", [P, CW // P], F32, kind="ExternalInput").ap()
    if use_k_bias:
        bk = nc.dram_tensor("bk", [P, CW // P], F32, kind="ExternalInput").ap()
    if use_v_bias:
        bv = nc.dram_tensor("bv", [P, CW], F32, kind="ExternalInput").ap()
    # [NST, P, 2, 512] has the same memory layout as [S, D] row-major
    out = nc.dram_tensor("out", [NST, P, 2, 512], FR, kind="ExternalOutput").ap()

    with tile.TileContext(nc) as tc, ExitStack() as ctx:
        wpool = ctx.enter_context(tc.tile_pool(name="wpool", bufs=1))
        xpool = ctx.enter_context(tc.tile_pool(name="xpool", bufs=7))
        qkpool = ctx.enter_context(tc.tile_pool(name="qkpool", bufs=1))
        ptpool = ctx.enter_context(tc.tile_pool(name="ptpool", bufs=8))
        smpool = ctx.enter_context(tc.tile_pool(name="smpool", bufs=4))
        outpool = ctx.enter_context(tc.tile_pool(name="outpool", bufs=4))
        mkpool = ctx.enter_context(tc.tile_pool(name="mkpool", bufs=3))
        # PSUM: pf (proj + final, 2x1 bank) + ps (scores pairs, 2x2 banks)
        # + po (attnV accum, 2x1 bank) = 8 banks exactly
        pf = ctx.enter_context(tc.tile_pool(name="pf", bufs=2, space="PSUM"))
        ps = ctx.enter_context(tc.tile_pool(name="ps", bufs=2, space="PSUM"))
        po = ctx.enter_context(tc.tile_pool(name="po", bufs=2, space="PSUM"))

        # --- persistent SBUF tensors (DMAs emitted lazily in the stream) ---
        wq_sb = wpool.tile([P, DC, CW], FR, tag="wq_sb")
        wk_sb = wpool.tile([P, DC, CW], FR, tag="wk_sb")
        wv_sb = wpool.tile([P, DC, CW], FR, tag="wv_sb")
        wo_sb = wpool.tile([P, CW // P, D], FR, tag="wo_sb")
        w_dma = {
            "q": lambda: nc.sync.dma_start(wq_sb[:], wq),
            "k": lambda: nc.sync.dma_start(wk_sb[:], wk),
            "v": lambda: nc.sync.dma_start(wv_sb[:], wv),
            "o": lambda: nc.sync.dma_start(wo_sb[:], wo),
        }
        mtri_sb = None
        if mode == "causal":
            mtri_sb = wpool.tile([P, P], F32, tag="mtri_sb")
        ones_v = wpool.tile([P, HG, 1], F32, tag="ones_v")
        nc.vector.memset(ones_v[:], 1.0)
        # touch Exp once so the ~2.7us activation-table load happens during
        # the startup DMA wait instead of on the first real softmax
        warm_act = wpool.tile([1, 1], F32, tag="warm_act")
        nc.scalar.activation(
            warm_act[:], ones_v[0:1, 0, :],
            mybir.ActivationFunctionType.Exp)
        bq_sb = bk_sb = bv_sb = None
        if use_q_bias:
            bq_sb = wpool.tile([P, CW // P], F32, tag="bq_sb")
            nc.sync.dma_start(bq_sb[:], bq)
        if use_k_bias:
            bk_sb = wpool.tile([P, CW // P], F32, tag="bk_sb")
            nc.sync.dma_start(bk_sb[:], bk)
        if use_v_bias:
            bv_sb = wpool.tile([P, CW], F32, tag="bv_sb")
            nc.sync.dma_start(bv_sb[:], bv)

        # Persistent per-block result tiles (fine-grained deps).
        QT_t = {}  # (cc, sb) -> [128, 512]
        KT_t = {}
        OT_t = {}
        for cc in range(CW // P):
            for sb in range(NSB):
                QT_t[(cc, sb)] = qkpool.tile(
                    [P, 512], FR, name=f"qt_{cc}_{sb}", tag=f"qt_{cc}_{sb}")
                KT_t[(cc, sb)] = qkpool.tile(
                    [P, 512], FR, name=f"kt_{cc}_{sb}", tag=f"kt_{cc}_{sb}")
                OT_t[(cc, sb)] = qkpool.tile(
                    [P, 512], FR, name=f"ot_{cc}_{sb}", tag=f"ot_{cc}_{sb}")
        V_t = {}  # st -> [128, HG, DEPTH+1] (ones col per head)
        for st in range(NST):
            V_t[st] = qkpool.tile(
                [P, HG, DEPTH + 1], FR, name=f"v_{st}", tag=f"v_{st}")
            # the ones column never changes: write it once at setup
            nc.vector.tensor_copy(
                V_t[st][:, :, DEPTH : DEPTH + 1], ones_v[:])

        slabs = {}  # (nm, sl) -> slab tile
        x_of = {"q": xq, "k": xk, "v": xv}

        def load_slab(nm, sl, halves=False):
            def _c():
                slab = xpool.tile([P, DC, 512], FR, tag="slab",
                                  name=f"sl{nm}_{sl}")
                if halves:
                    # two half-slab DMAs: the first 4 dc chunks arrive early
                    # so the projection's accumulation chain starts sooner
                    h = DC // 2
                    nc.sync.dma_start(slab[:, 0:h, :], x_of[nm][sl][:, 0:h, :])
                    nc.sync.dma_start(slab[:, h:, :], x_of[nm][sl][:, h:, :])
                else:
                    nc.sync.dma_start(slab[:], x_of[nm][sl])
                slabs[(nm, sl)] = slab
            return _c

        def proj_psum(pool_sel, name):
            # a [P,512] accumulator from either psum pool; the "ps" pool's
            # slot is 2 banks wide, use its first bank
            if pool_sel == "ps":
                t = ps.tile([P, 2, 512], F32, tag="ps", name=name)
                return t[:, 0, :]
            return pf.tile([P, 512], F32, tag="pf", name=name)

        def v_group(sl, sq, pool_sel="pf"):
            def _c():
                st = sl * 4 + sq
                slab = slabs[("v", sl)]
                psum_v = proj_psum(pool_sel, f"pv_{st}")
                for dc in range(DC):
                    nc.tensor.matmul(
                        psum_v[:, :CW],
                        lhsT=slab[:, dc, ts(sq, P)],
                        rhs=wv_sb[:, dc, :],
                        start=(dc == 0),
                        stop=(dc == DC - 1),
                    )
                psrc = psum_v[:, :CW].rearrange("p (h d) -> p h d", h=HG)
                if use_v_bias:
                    nc.vector.tensor_tensor(
                        V_t[st][:, :, 0:DEPTH], psrc,
                        bv_sb.rearrange("p (h d) -> p h d", h=HG),
                        mybir.AluOpType.add,
                    )
                else:
                    nc.vector.tensor_copy(V_t[st][:, :, 0:DEPTH], psrc)
            return _c

        def qk_group(nm, cc, sl, pool_sel="pf"):
            w_sb, b_sb, T_t = {
                "q": (wq_sb, bq_sb, QT_t),
                "k": (wk_sb, bk_sb, KT_t),
            }[nm]

            def _c():
                slab = slabs[(nm, sl)]
                psum_q = proj_psum(pool_sel, f"p{nm}_{cc}_{sl}")
                for dc in range(DC):
                    nc.tensor.matmul(
                        psum_q[:],
                        lhsT=w_sb[:, dc, ts(cc, P)],
                        rhs=slab[:, dc, :],
                        start=(dc == 0),
                        stop=(dc == DC - 1),
                    )
                if b_sb is not None:
                    nc.vector.tensor_scalar_add(
                        T_t[(cc, sl)][:], psum_q[:], b_sb[:, cc : cc + 1])
                else:
                    nc.vector.tensor_copy(T_t[(cc, sl)][:], psum_q[:])
            return _c

        def attention_block(i, inject=(), l_on_scalar=()):
            """inject: list of (pos, chunk); chunk emitted when the unit
            counter (cc-major over j) reaches pos. l_on_scalar: cc values
            whose softmax-denominator copies go on ScalarE (tail units,
            where the Vector queue is the bottleneck)."""
            inject = sorted(inject, key=lambda t: t[0])
            inj_idx = 0
            jmax = 4 * i + 4 if mode == "causal" else NST
            jcount = 0
            for cc in range(CW // P):  # head pair (2cc, 2cc+1)
                po0 = po.tile([DEPTH + 1, 512], F32, tag="po",
                              name=f"po0_{i}_{cc}")
                po1 = po.tile([DEPTH + 1, 512], F32, tag="po",
                              name=f"po1_{i}_{cc}")
                pos = (po0, po1)
                for j in range(jmax):
                    while inj_idx < len(inject) and inject[inj_idx][0] <= jcount:
                        inject[inj_idx][1]()
                        inj_idx += 1
                    r = j - 4 * i
                    lo = P * r if (mode == "causal" and r >= 0) else 0
                    psj = ps.tile([P, 2, 512], F32, tag="ps",
                                  name=f"ps_{i}_{cc}_{j}")
                    for hh in range(2):
                        nc.tensor.matmul(
                            psj[:, hh, lo:],
                            lhsT=KT_t[(cc, j // 4)][
                                DEPTH * hh : DEPTH * hh + DEPTH, ts(j % 4, P)],
                            rhs=QT_t[(cc, i)][DEPTH * hh : DEPTH * hh + DEPTH, lo:],
                            start=True,
                            stop=True,
                        )
                    if mode == "generic":
                        mk = mkpool.tile([P, 512], F32, tag="mk",
                                         name=f"mk_{i}_{cc}_{j}")
                        nc.sync.dma_start(mk[:], mneg[ts(j, P), ts(i, 512)])
                        nc.vector.tensor_tensor(
                            psj[:], psj[:],
                            mk[:, None, :].to_broadcast((P, 2, 512)),
                            mybir.AluOpType.add,
                        )
                    pt = ptpool.tile([P, 2, 512], FR, tag="pt",
                                     name=f"pt_{i}_{cc}_{j}")
                    nc.scalar.activation(
                        pt[:, :, lo:],
                        psj[:, :, lo:],
                        mybir.ActivationFunctionType.Exp,
                        scale=SCALE,
                    )
                    if mode == "causal" and r >= 0:
                        # zero the below-diagonal strip of exp's output
                        nc.vector.tensor_tensor(
                            pt[:, :, lo : lo + P],
                            pt[:, :, lo : lo + P],
                            mtri_sb[:, None, :].to_broadcast((P, 2, P)),
                            mybir.AluOpType.mult,
                        )
                    for hh in range(2):
                        nc.tensor.matmul(
                            pos[hh][:, lo:],
                            lhsT=V_t[j][:, 2 * cc + hh, :],
                            rhs=pt[:, hh, lo:],
                            start=(j == 0),
                            stop=(j == jmax - 1),
                        )
                    jcount += 1
                # normalize both heads: OT[c, q] = outT[c, q] / l[q].
                # l staged to SBUF (custom-DVE recip can't read PSUM); the
                # final scale reads the PSUM accumulator directly.
                l_sb = {}
                rl_sb = {}
                rb = {}
                for hh in range(2):
                    l_sb[hh] = smpool.tile([1, 512], F32, tag="l_sb",
                                           name=f"l_{i}_{cc}_{hh}")
                    if cc in l_on_scalar:
                        nc.scalar.copy(
                            l_sb[hh][:], pos[hh][DEPTH : DEPTH + 1, :])
                    else:
                        nc.vector.tensor_copy(
                            l_sb[hh][:], pos[hh][DEPTH : DEPTH + 1, :])
                for hh in range(2):
                    rl_sb[hh] = smpool.tile([1, 512], F32, tag="rl_sb",
                                            name=f"rl_{i}_{cc}_{hh}")
                    nc.vector.reciprocal_approx_fast(
                        out=rl_sb[hh][:], in_=l_sb[hh][:])
                for hh in range(2):
                    rb[hh] = smpool.tile([DEPTH, 512], F32, tag="rb",
                                         name=f"rb_{i}_{cc}_{hh}")
                    nc.gpsimd.partition_broadcast(rb[hh][:], rl_sb[hh][:])
                for hh in range(2):
                    nc.vector.tensor_tensor(
                        OT_t[(cc, i)][DEPTH * hh : DEPTH * hh + DEPTH, :],
                        pos[hh][0:DEPTH, :],
                        rb[hh][:],
                        mybir.AluOpType.mult,
                    )

            while inj_idx < len(inject):
                inject[inj_idx][1]()
                inj_idx += 1

        def output_chunks(i, split_engines=False, alt_pool=False):
            """8 chunks; each = 2 accumulating fin MMs + copy; one DMA per
            qq once both eh halves are in SBUF."""
            chunks = []
            out_ts = {}

            def fin_group(qq, eh, i=i):
                def _c():
                    qt = 4 * i + qq
                    psum_f = proj_psum(
                        "ps" if (alt_pool and eh == 0) else "pf",
                        f"pfin_{qt}_{eh}")
                    for cc2 in range(CW // P):
                        nc.tensor.matmul(
                            psum_f[:],
                            lhsT=OT_t[(cc2, i)][:, ts(qq, P)],
                            rhs=wo_sb[:, cc2, ts(eh, 512)],
                            start=(cc2 == 0),
                            stop=(cc2 == CW // P - 1),
                        )
                    if eh == 0:
                        out_ts[qq] = outpool.tile(
                            [P, 2, 512], FR, tag="out_t", name=f"ot_{qt}")
                    out_t = out_ts[qq]
                    if split_engines and eh == 1:
                        nc.scalar.copy(out_t[:, eh, :], psum_f[:])
                    else:
                        nc.vector.tensor_copy(out_t[:, eh, :], psum_f[:])
                    if eh == 1:
                        nc.sync.dma_start(out[qt], out_t[:])
                return _c

            for qq in range(4):
                for eh in range(2):
                    chunks.append(fin_group(qq, eh))
            return chunks

        def output_block(i, split_engines=False, alt_pool=False):
            for c in output_chunks(i, split_engines, alt_pool):
                c()

        if mode == "causal":
            # -- pre-phase: minimum projections for attention block 1 --
            w_dma["k"]()
            load_slab("k", 0, halves=True)()
            w_dma["q"]()
            load_slab("q", 1, halves=True)()
            w_dma["v"]()
            load_slab("v", 0)()
            nc.sync.dma_start(mtri_sb[:], mtri)
            qk_group("k", 0, 0, "ps")()
            qk_group("k", 1, 0, "pf")()
            qk_group("q", 0, 1, "ps")()
            qk_group("q", 1, 1, "pf")()
            for sq in range(4):
                v_group(0, sq, "ps" if sq % 2 == 0 else "pf")()

            # -- attention phases with positioned PE filler --
            out1 = output_chunks(1)
            out2 = output_chunks(2)
            attention_block(1, inject=[
                (0, load_slab("k", 1)), (0, load_slab("v", 1)),
                (1, qk_group("k", 0, 1)), (2, qk_group("k", 1, 1)),
                (3, v_group(1, 0)), (4, v_group(1, 1)),
                (5, v_group(1, 2)), (6, v_group(1, 3)),
                (7, load_slab("q", 2)), (8, w_dma["o"]),
                (9, load_slab("k", 2)), (11, load_slab("v", 2)),
                (13, qk_group("q", 0, 2)),
            ])
            attention_block(2, inject=[
                (0, qk_group("q", 1, 2)),
                (1, qk_group("k", 0, 2)), (3, qk_group("k", 1, 2)),
                (4, v_group(2, 0)), (5, v_group(2, 1)),
                (6, v_group(2, 2)), (7, v_group(2, 3)),
                (9, load_slab("q", 3)), (10, qk_group("q", 0, 3)),
                (12, qk_group("q", 1, 3)),
                (14, load_slab("k", 3)), (19, load_slab("v", 3)),
            ])
            attention_block(3, inject=[
                (0, load_slab("q", 0)),
                (1, qk_group("k", 0, 3)), (3, qk_group("k", 1, 3)),
                (2, v_group(3, 0)), (4, v_group(3, 1)),
                (6, v_group(3, 2)), (8, v_group(3, 3)),
                (10, qk_group("q", 0, 0)), (12, qk_group("q", 1, 0)),
                (14, out1[0]), (16, out1[1]), (18, out1[2]),
                (20, out1[3]), (22, out1[4]), (24, out1[5]),
                (26, out1[6]), (28, out1[7]),
            ], l_on_scalar=(1,))
            # out2's fins pinned at the att3->att0 po-ring stall (position
            # 0): the PE blocks there on att3's last normalize, and filler
            # behind the blocked attnV in the queue cannot run. out3's fins
            # (ready once att3's normalize lands) cover att0's second half
            # and block 0's own normalize chains.
            out3 = output_chunks(3, split_engines=True, alt_pool=True)
            attention_block(0, inject=[
                (0, out2[0]), (0, out2[1]), (0, out2[2]), (0, out2[3]),
                (0, out2[4]), (0, out2[5]), (0, out2[6]), (0, out2[7]),
                (4, out3[0]), (4, out3[1]), (5, out3[2]), (5, out3[3]),
                (6, out3[4]), (6, out3[5]), (7, out3[6]), (7, out3[7]),
            ], l_on_scalar=(0, 1))
            # tail: only block 0's own output projection remains
            output_block(0, split_engines=True, alt_pool=True)
        else:
            # dense/generic need all KT/V before any attention block
            w_dma["o"]()
            for nm in ("v", "q", "k"):
                w_dma[nm]()
            for sl in range(NSB):
                load_slab("v", sl)()
                for sq in range(4):
                    v_group(sl, sq)()
                load_slab("q", sl)()
                for cc in range(CW // P):
                    qk_group("q", cc, sl)()
                load_slab("k", sl)()
                for cc in range(CW // P):
                    qk_group("k", cc, sl)()
            for i in range(NSB):
                attention_block(i)
                output_block(i)

    nc.compile()
    return nc


_PROG_CACHE = {}


def _get_program(mode, use_q_bias, use_k_bias, use_v_bias):
    key = (mode, use_q_bias, use_k_bias, use_v_bias)
    if key not in _PROG_CACHE:
        _PROG_CACHE[key] = _build_program(mode, use_q_bias, use_k_bias, use_v_bias)
    return _PROG_CACHE[key]


import ml_dtypes


def _pretile(x2d):
    # [S, D] -> [NSB, P, DC, 512]: arr[sl, p, dc, s] = x2d[sl*512+s, dc*128+p]
    return np.ascontiguousarray(
        x2d.reshape(NSB, 512, DC, P).transpose(0, 3, 2, 1)
    ).astype(ml_dtypes.bfloat16)


def _pretile_w(w):
    # [D, CW] -> [P, DC, CW]
    return np.ascontiguousarray(
        w.reshape(DC, P, CW).transpose(1, 0, 2)).astype(ml_dtypes.bfloat16)


def kernel(**inputs):
    query = np.asarray(inputs["query"], np.float32)
    key = np.asarray(inputs["key"], np.float32)
    value = np.asarray(inputs["value"], np.float32)
    mask = np.asarray(inputs["mask"], np.float32).reshape(S, S)
    wq = np.asarray(inputs["wq"], np.float32)
    wk = np.asarray(inputs["wk"], np.float32)
    wv = np.asarray(inputs["wv"], np.float32)
    wo = np.asarray(inputs["wo"], np.float32)
    bq = np.asarray(inputs["bq"], np.float32)
    bk = np.asarray(inputs["bk"], np.float32)
    bv = np.asarray(inputs["bv"], np.float32)
    bo = np.asarray(inputs["bo"], np.float32)

    if not mask.any():
        mode = "dense"
    elif np.array_equal(mask, np.triu(np.ones((S, S), np.float32), 1)):
        mode = "causal"
    else:
        mode = "generic"
    use_q_bias = bool(bq.any())
    use_k_bias = bool(bk.any())
    use_v_bias = bool(bv.any())

    nc = _get_program(mode, use_q_bias, use_k_bias, use_v_bias)

    in_maps = []
    for core in range(NCORES):
        b, g = core // GROUPS, core % GROUPS
        cs = slice(g * CW, (g + 1) * CW)
        m = {
            "xq": _pretile(query[b]),
            "xk": _pretile(key[b]),
            "xv": _pretile(value[b]),
            "wq": _pretile_w(wq[:, cs]),
            "wk": _pretile_w(wk[:, cs]),
            "wv": _pretile_w(wv[:, cs]),
            "wo": np.ascontiguousarray(
                wo[cs, :].reshape(CW // P, P, D).transpose(1, 0, 2)
            ).astype(ml_dtypes.bfloat16),
        }
        if mode == "causal":
            m["mtri01"] = np.triu(np.ones((P, P), np.float32), 0)
        elif mode == "generic":
            m["mneg"] = np.ascontiguousarray(mask.T) * NEG
        if use_q_bias:
            m["bq"] = np.ascontiguousarray(bq[cs].reshape(CW // P, P).T)
        if use_k_bias:
            m["bk"] = np.ascontiguousarray(bk[cs].reshape(CW // P, P).T)
        if use_v_bias:
            m["bv"] = np.ascontiguousarray(np.tile(bv[cs], (P, 1)))
        in_maps.append(m)

    res = bass_utils.run_bass_kernel_spmd(
        nc, in_maps, core_ids=list(range(NCORES)), trace=False
    )
    outs = [np.asarray(r["out"], np.float32).reshape(S, D) for r in res.results]
    full = np.empty((B, S, D), np.float32)
    for b in range(B):
        full[b] = outs[GROUPS * b]
        for g in range(1, GROUPS):
            full[b] += outs[GROUPS * b + g]
        full[b] += bo
    return full
